# revision 17
# baseline (speedup 1.0000x reference)
"""Focal-loss + smooth-L1 loss kernel for TRN2, SPMD over 8 NeuronCores.

Sharding: data-parallel over the batch axis (B=8 -> one batch row per core).

Host prep (per core), all bf16:
  conf16 [A, 81]  - logits
  aux   [A, 12]   - loc(4), box(4), lab, labq=lab//9, labr=lab-9*labq,
                    xsel=conf[n, max(lab,0)]
Device (per core, anchor n = 600*p + t; tiles of T=75, last tile P=127):
  phase A per tile (pipelined):
    e[:, :, 0:81] = exp(conf)       (scalar engine; e rows padded to 96 with
                                     persistent zero pad cols for the fold)
    s = fold-tree sum_c e (96->48->24->12->6->3->reduce)  (vector, 2x bf16)
    aq -> rhs_all[.., 9:18], ar -> ar_all, xsel -> xsel_all
    smooth-L1 partials on gpsimd, strip reduce on vector
  phase B per quad of tiles (batches ACT table switches):
    lns = ln(s); pt = exp(xsel - lns); w0 = (1-pt)^2*(lns-xsel)
    rhs_all[.., 0:9] = aq * w0
    per-t matmul ph[r, k] += ar_t^T @ rhs_t -> PSUM [9, 18]
Host combine: h[9q+r] = ph[r, q], cnt[9q+r] = ph[r, 9+q]; tiny final math.

All bulk HBM->SBUF transfers go through SWDGE (gpsimd) so descriptors
spread across all 16 SDMA engines (HWDGE pins them to one engine).
"""

import numpy as np
import ml_dtypes

import concourse.bass as bass
import concourse.bacc as bacc
import concourse.mybir as mybir
import concourse.tile as tile
from concourse.bass_utils import run_bass_kernel_spmd

BF16NP = np.dtype(ml_dtypes.bfloat16)

F32 = mybir.dt.float32
BF16 = mybir.dt.bfloat16
I16 = mybir.dt.int16
AF = mybir.ActivationFunctionType
OP = mybir.AluOpType
AX = mybir.AxisListType

C = 81
CP = 96  # padded e-row width (even fold widths: 96/48/24/12/6/3)
Q = 9    # base-9 split: class c = 9*q + r
QUAD = 4  # tiles per phase-B batch


def build_kernel(A, APP, T):
    n_tiles = APP // T
    t_full = A - 127 * APP
    assert t_full == (n_tiles - 1) * T, (A, APP, T, t_full)

    nc = bacc.Bacc(None, target_bir_lowering=False)
    conf = nc.dram_tensor("conf", [A, C], BF16, kind="ExternalInput")
    aux = nc.dram_tensor("aux", [A, 12], BF16, kind="ExternalInput")
    hist = nc.dram_tensor("hist", [Q, 2 * Q], F32, kind="ExternalOutput")
    locs = nc.dram_tensor("locs", [128, 1], F32, kind="ExternalOutput")

    with tile.TileContext(nc) as tc:
        with (
            tc.tile_pool(name="singles", bufs=1) as singles,
            tc.tile_pool(name="io", bufs=4) as io,
            tc.tile_pool(name="small", bufs=3) as small,
            tc.tile_pool(name="psum", bufs=1, space="PSUM") as psum,
        ):
            # constants / persistent accumulators
            iota_i = singles.tile([128, Q], I16)
            nc.gpsimd.iota(iota_i[:, :], [[1, Q]], channel_multiplier=0)
            iota9 = singles.tile([128, Q], BF16)
            nc.vector.tensor_copy(iota9[:, :], iota_i[:, :])
            strip = singles.tile([128, n_tiles], F32)
            nc.vector.memset(strip[:, :], 0.0)

            NEB = 3
            e_bufs = [
                singles.tile([128, T, CP], BF16, tag=f"e{k}", name=f"e{k}")
                for k in range(NEB)
            ]
            for k in range(NEB):
                nc.vector.memset(e_bufs[k][:, :, C:CP], 0.0)

            s_all = singles.tile([128, APP], F32)
            nc.vector.memset(s_all[:, :], 1.0)
            xsel_all = singles.tile([128, APP], BF16)
            u_all = singles.tile([128, APP], F32)
            lns_all = singles.tile([128, APP], F32)
            pt_all = singles.tile([128, APP], F32)
            pm1_all = singles.tile([128, APP], F32)
            usq_all = singles.tile([128, APP], F32)
            w0_all = singles.tile([128, APP], BF16)
            rhs_all = singles.tile([128, APP, 2 * Q], BF16)
            ar_all = singles.tile([128, APP, Q], BF16)

            NB = 8  # PSUM banks round-robin to avoid accumulation RAW serialization
            ph_banks = [
                psum.tile([Q, 2 * Q], F32, tag=f"ph{k}", name=f"ph{k}")
                for k in range(NB)
            ]

            def phase_b(qi):
                """Ln/exp batch + matmuls for tiles qi*QUAD .. qi*QUAD+QUAD-1."""
                g0 = qi * QUAD * T
                g1 = min((qi + 1) * QUAD * T, APP)
                W = g1 - g0
                sl = slice(g0, g1)
                nc.scalar.activation(lns_all[:, sl], s_all[:, sl], AF.Ln)
                nc.vector.tensor_tensor(
                    u_all[:, sl], xsel_all[:, sl], lns_all[:, sl], OP.subtract
                )
                nc.scalar.activation(pt_all[:, sl], u_all[:, sl], AF.Exp)
                nc.vector.tensor_scalar_add(pm1_all[:, sl], pt_all[:, sl], -1.0)
                nc.vector.tensor_tensor(
                    usq_all[:, sl], pm1_all[:, sl], pm1_all[:, sl], OP.mult
                )
                nc.vector.scalar_tensor_tensor(
                    w0_all[:, sl], u_all[:, sl], -1.0, usq_all[:, sl],
                    OP.mult, OP.mult,
                )  # w0 = (lns - xsel) * (1-pt)^2
                nc.vector.tensor_tensor(
                    rhs_all[:, sl, 0:Q],
                    rhs_all[:, sl, Q : 2 * Q],
                    w0_all[:, sl, None].broadcast_to([128, W, Q]),
                    OP.mult,
                )
                for tg in range(g0, g1):
                    P = 128 if tg < t_full else 127
                    nc.tensor.matmul(
                        ph_banks[tg % NB][:, :],
                        ar_all[:P, tg, :],
                        rhs_all[:P, tg, :],
                        start=(tg < NB),
                        stop=(tg >= APP - NB),
                    )

            PREFETCH = 3
            pending = {}

            def dispatch(j):
                jt0 = j * T
                jP = 128 if j < n_tiles - 1 else 127
                conf_t = io.tile([128, T, C], BF16, tag="conf", name="conf_t")
                nc.gpsimd.dma_start(
                    conf_t[:jP],
                    bass.AP(
                        tensor=conf[:, :].tensor,
                        offset=jt0 * C,
                        ap=[[APP * C, jP], [C, T], [1, C]],
                    ),
                )
                aux_t = io.tile([128, T, 12], BF16, tag="aux", name="aux_t")
                nc.gpsimd.dma_start(
                    aux_t[:jP],
                    bass.AP(
                        tensor=aux[:, :].tensor,
                        offset=jt0 * 12,
                        ap=[[APP * 12, jP], [12, T], [1, 12]],
                    ),
                )
                pending[j] = (conf_t, aux_t)

            for j in range(PREFETCH):
                dispatch(j)

            for i in range(n_tiles):
                t0 = i * T
                P = 128 if i < n_tiles - 1 else 127
                ts = slice(t0, t0 + T)

                conf_t, aux_t = pending.pop(i)

                lab = aux_t[:P, :, 8:9]
                labq = aux_t[:P, :, 9:10]
                labr = aux_t[:P, :, 10:11]

                # ---- conf path ----
                e_t = e_bufs[i % 2]
                nc.scalar.activation(e_t[:P, :, 0:C], conf_t[:P], AF.Exp)
                w = CP
                while w > 3:
                    h = w // 2
                    nc.vector.tensor_tensor(
                        e_t[:P, :, 0:h], e_t[:P, :, 0:h], e_t[:P, :, h:w], OP.add
                    )
                    w = h
                nc.vector.reduce_sum(s_all[:P, ts], e_t[:P, :, 0:3], axis=AX.X)

                nc.vector.tensor_copy(xsel_all[:P, ts], aux_t[:P, :, 11:12].squeeze())

                nc.vector.tensor_tensor(
                    rhs_all[:P, ts, Q : 2 * Q],
                    iota9[:P, None, :].broadcast_to([P, T, Q]),
                    labq.broadcast_to([P, T, Q]),
                    OP.is_equal,
                )
                nc.vector.tensor_tensor(
                    ar_all[:P, ts, :],
                    iota9[:P, None, :].broadcast_to([P, T, Q]),
                    labr.broadcast_to([P, T, Q]),
                    OP.is_equal,
                )

                # ---- loc path on gpsimd (bf16), strip reduce on vector ----
                df = small.tile([128, T, 4], BF16, tag="df")
                nc.gpsimd.tensor_tensor(
                    df[:P], aux_t[:P, :, 0:4], aux_t[:P, :, 4:8], OP.subtract
                )
                ad = small.tile([128, T, 4], BF16, tag="ad")
                nc.vector.scalar_tensor_tensor(
                    ad[:P], df[:P], -1.0, df[:P], OP.mult, OP.max
                )
                dm = small.tile([128, T, 4], BF16, tag="dm")
                nc.vector.tensor_scalar_min(dm[:P], ad[:P], 1.0)
                r_t = small.tile([128, T, 4], BF16, tag="r")
                nc.gpsimd.tensor_tensor(r_t[:P], ad[:P], dm[:P], OP.subtract)
                q_t = small.tile([128, T, 4], BF16, tag="q")
                nc.gpsimd.tensor_tensor(q_t[:P], dm[:P], dm[:P], OP.mult)
                sl1 = small.tile([128, T, 4], BF16, tag="sl1")
                nc.vector.scalar_tensor_tensor(
                    sl1[:P], q_t[:P], 0.5, r_t[:P], OP.mult, OP.add
                )
                pos = small.tile([128, T], BF16, tag="pos")
                nc.vector.tensor_scalar(pos[:P], lab.squeeze(), 0.0, None, OP.is_gt)
                slm = small.tile([128, T, 4], BF16, tag="slm")
                nc.gpsimd.tensor_tensor(
                    slm[:P], sl1[:P], pos[:P, :, None].broadcast_to([P, T, 4]), OP.mult
                )
                with nc.allow_low_precision("partial sums accumulate in f32 strip"):
                    nc.vector.reduce_sum(strip[:P, i : i + 1], slm[:P], axis=AX.XY)

                if i + PREFETCH < n_tiles:
                    dispatch(i + PREFETCH)

                if (i + 1) % QUAD == 0:
                    phase_b((i + 1) // QUAD - 1)

            # ---- finalize ----
            hc = singles.tile([Q, 2 * Q], F32)
            nc.vector.tensor_copy(hc[:, :], ph_banks[0][:, :])
            for k in range(1, NB):
                nc.vector.tensor_tensor(hc[:, :], hc[:, :], ph_banks[k][:, :], OP.add)
            nc.sync.dma_start(hist[:, :], hc[:, :])
            lacc = singles.tile([128, 1], F32)
            nc.vector.reduce_sum(lacc[:, :], strip[:, :], axis=AX.X)
            nc.sync.dma_start(locs[:, :], lacc[:, :])

    nc.compile()
    return nc


_CACHED = {}


def _get_nc(A, APP, T):
    key = (A, APP, T)
    if key not in _CACHED:
        _CACHED[key] = build_kernel(A, APP, T)
    return _CACHED[key]


def combine_host(hists, locsums, alpha):
    """hists: [ncores, 9, 18]; locsums: [ncores, 128, 1]; alpha: [81]."""
    hw = hists[:, :, 0:Q].sum(axis=0).astype(np.float64)       # [r, q]
    hcnt = hists[:, :, Q : 2 * Q].sum(axis=0).astype(np.float64)
    h = hw.T.ravel()[:C]      # h[9q+r]
    cnt = hcnt.T.ravel()[:C]
    alpha = alpha.astype(np.float64)
    denom = np.clip(alpha * cnt, 1.0, None)
    conf_loss = np.sum(alpha * h / denom)
    num_pos = cnt[1:].sum()
    loc_sum = locsums.astype(np.float64).sum()
    denom_loc = max(num_pos * 4.0, 1.0)
    loc_loss = loc_sum / denom_loc if num_pos > 0 else 0.0
    return np.float32(loc_loss), np.float32(conf_loss)


def kernel(loc_pred, conf_pred, targets, alpha, _trace=False):
    B, A, _ = conf_pred.shape
    assert B == 8 and A == 76725
    nc = _get_nc(A, 600, 75)

    conf16 = np.asarray(conf_pred, dtype=BF16NP)               # [B, A, 81]
    tgt = np.asarray(targets, dtype=np.float32)
    lab_i = tgt[:, :, 4].astype(np.int32)                      # [B, A]
    labq = lab_i // 9
    labr = lab_i - 9 * labq
    xsel = np.take_along_axis(
        conf16, np.maximum(lab_i, 0)[:, :, None], axis=2
    )[:, :, 0]                                                 # [B, A] bf16

    aux = np.empty((B, A, 12), dtype=BF16NP)
    aux[:, :, 0:4] = loc_pred
    aux[:, :, 4:8] = tgt[:, :, 0:4]
    aux[:, :, 8] = tgt[:, :, 4]
    aux[:, :, 9] = labq
    aux[:, :, 10] = labr
    aux[:, :, 11] = xsel

    in_maps = [
        {
            "conf": np.ascontiguousarray(conf16[b]),
            "aux": np.ascontiguousarray(aux[b]),
        }
        for b in range(B)
    ]
    res = run_bass_kernel_spmd(nc, in_maps, core_ids=list(range(B)), trace=_trace)
    hists = np.stack([r["hist"] for r in res.results])
    locsums = np.stack([r["locs"] for r in res.results])
    out = combine_host(hists, locsums, np.asarray(alpha, dtype=np.float32))
    if _trace:
        return out, res
    return out


# revision 21
# speedup vs baseline: 1.3580x; 1.3580x over previous
"""Focal-loss + smooth-L1 loss kernel for TRN2, SPMD over 8 NeuronCores.

Sharding: data-parallel over the batch axis (B=8 -> one batch row per core).

Host prep (per core), all bf16:
  conf16 [A, 81]  - logits
  aux   [A, 12]   - loc(4), box(4), lab, labq=lab//9, labr=lab-9*labq,
                    xsel=conf[n, max(lab,0)]
Device (per core, anchor n = 600*p + t; tiles of T=75, last tile P=127):
  phase A per tile (pipelined):
    e[:, :, 0:81] = exp(conf)       (scalar engine; e rows padded to 96 with
                                     persistent zero pad cols for the fold)
    s = fold-tree sum_c e (96->48->24->12->6->3->reduce)  (vector, 2x bf16)
    aq -> rhs_all[.., 9:18], ar -> ar_all, xsel -> xsel_all
    smooth-L1 partials on gpsimd, strip reduce on vector
  phase B per quad of tiles (batches ACT table switches):
    lns = ln(s); pt = exp(xsel - lns); w0 = (1-pt)^2*(lns-xsel)
    rhs_all[.., 0:9] = aq * w0
    per-t matmul ph[r, k] += ar_t^T @ rhs_t -> PSUM [9, 18]
Host combine: h[9q+r] = ph[r, q], cnt[9q+r] = ph[r, 9+q]; tiny final math.

All bulk HBM->SBUF transfers go through SWDGE (gpsimd) so descriptors
spread across all 16 SDMA engines (HWDGE pins them to one engine).
"""

import numpy as np
import ml_dtypes

import concourse.bass as bass
import concourse.bacc as bacc
import concourse.mybir as mybir
import concourse.tile as tile
from concourse.bass_utils import run_bass_kernel_spmd

BF16NP = np.dtype(ml_dtypes.bfloat16)

F32 = mybir.dt.float32
BF16 = mybir.dt.bfloat16
I16 = mybir.dt.int16
AF = mybir.ActivationFunctionType
OP = mybir.AluOpType
AX = mybir.AxisListType

C = 81
CP = 96  # padded e-row width (even fold widths: 96/48/24/12/6/3)
Q = 9    # base-9 split: class c = 9*q + r
QUAD = 4  # tiles per phase-B batch


def build_kernel(A, APP, T):
    """A anchors (padded so A == 128*APP -> every tile uses all 128 partitions)."""
    n_tiles = APP // T
    assert A == 128 * APP, (A, APP)

    nc = bacc.Bacc(None, target_bir_lowering=False)
    conf = nc.dram_tensor("conf", [A, C], BF16, kind="ExternalInput")
    aux = nc.dram_tensor("aux", [A, 12], BF16, kind="ExternalInput")
    hist = nc.dram_tensor("hist", [Q, 2 * Q], F32, kind="ExternalOutput")
    locs = nc.dram_tensor("locs", [128, 1], F32, kind="ExternalOutput")

    with tile.TileContext(nc) as tc:
        with (
            tc.tile_pool(name="singles", bufs=1) as singles,
            tc.tile_pool(name="io", bufs=4) as io,
            tc.tile_pool(name="small", bufs=3) as small,
            tc.tile_pool(name="psum", bufs=1, space="PSUM") as psum,
        ):
            # constants / persistent accumulators
            iota_i = singles.tile([128, Q], I16)
            nc.gpsimd.iota(iota_i[:, :], [[1, Q]], channel_multiplier=0)
            iota9 = singles.tile([128, Q], BF16)
            nc.vector.tensor_copy(iota9[:, :], iota_i[:, :])
            strip = singles.tile([128, n_tiles], F32)
            nc.vector.memset(strip[:, :], 0.0)

            NEB = 3
            e_bufs = [
                singles.tile([128, T, CP], BF16, tag=f"e{k}", name=f"e{k}")
                for k in range(NEB)
            ]
            for k in range(NEB):
                nc.vector.memset(e_bufs[k][:, :, C:CP], 0.0)

            s_all = singles.tile([128, APP], F32)
            nc.vector.memset(s_all[:, :], 1.0)
            xsel_all = singles.tile([128, APP], BF16)
            u_all = singles.tile([128, APP], F32)
            lns_all = singles.tile([128, APP], F32)
            pt_all = singles.tile([128, APP], F32)
            pm1_all = singles.tile([128, APP], F32)
            usq_all = singles.tile([128, APP], F32)
            w0_all = singles.tile([128, APP], BF16)
            rhs_all = singles.tile([128, APP, 2 * Q], BF16)
            ar_all = singles.tile([128, APP, Q], BF16)

            NB = 8  # PSUM banks round-robin to avoid accumulation RAW serialization
            ph_banks = [
                psum.tile([Q, 2 * Q], F32, tag=f"ph{k}", name=f"ph{k}")
                for k in range(NB)
            ]

            def phase_b(qi):
                """Ln/exp batch + matmuls for tiles qi*QUAD .. qi*QUAD+QUAD-1."""
                g0 = qi * QUAD * T
                g1 = min((qi + 1) * QUAD * T, APP)
                W = g1 - g0
                sl = slice(g0, g1)
                nc.scalar.activation(lns_all[:, sl], s_all[:, sl], AF.Ln)
                nc.vector.tensor_tensor(
                    u_all[:, sl], xsel_all[:, sl], lns_all[:, sl], OP.subtract
                )
                nc.scalar.activation(pt_all[:, sl], u_all[:, sl], AF.Exp)
                nc.vector.tensor_scalar_add(pm1_all[:, sl], pt_all[:, sl], -1.0)
                nc.vector.tensor_tensor(
                    usq_all[:, sl], pm1_all[:, sl], pm1_all[:, sl], OP.mult
                )
                nc.vector.scalar_tensor_tensor(
                    w0_all[:, sl], u_all[:, sl], -1.0, usq_all[:, sl],
                    OP.mult, OP.mult,
                )  # w0 = (lns - xsel) * (1-pt)^2
                nc.vector.tensor_tensor(
                    rhs_all[:, sl, 0:Q],
                    rhs_all[:, sl, Q : 2 * Q],
                    w0_all[:, sl, None].broadcast_to([128, W, Q]),
                    OP.mult,
                )
                for tg in range(g0, g1):
                    nc.tensor.matmul(
                        ph_banks[tg % NB][:, :],
                        ar_all[:, tg, :],
                        rhs_all[:, tg, :],
                        start=(tg < NB),
                        stop=(tg >= APP - NB),
                    )

            PREFETCH = 3
            pending = {}

            def dispatch(j):
                jt0 = j * T
                conf_t = io.tile([128, T, C], BF16, tag="conf", name="conf_t")
                nc.gpsimd.dma_start(
                    conf_t[:, :, :],
                    bass.AP(
                        tensor=conf[:, :].tensor,
                        offset=jt0 * C,
                        ap=[[APP * C, 128], [C, T], [1, C]],
                    ),
                )
                aux_t = io.tile([128, T, 12], BF16, tag="aux", name="aux_t")
                nc.gpsimd.dma_start(
                    aux_t[:, :, :],
                    bass.AP(
                        tensor=aux[:, :].tensor,
                        offset=jt0 * 12,
                        ap=[[APP * 12, 128], [12, T], [1, 12]],
                    ),
                )
                pending[j] = (conf_t, aux_t)

            for j in range(PREFETCH):
                dispatch(j)

            for i in range(n_tiles):
                t0 = i * T
                P = 128
                ts = slice(t0, t0 + T)

                conf_t, aux_t = pending.pop(i)

                lab = aux_t[:P, :, 8:9]
                labq = aux_t[:P, :, 9:10]
                labr = aux_t[:P, :, 10:11]

                # ---- conf path ----
                e_t = e_bufs[i % NEB]
                nc.scalar.activation(e_t[:P, :, 0:C], conf_t[:P], AF.Exp)
                w = CP
                while w > 3:
                    h = w // 2
                    nc.vector.tensor_tensor(
                        e_t[:P, :, 0:h], e_t[:P, :, 0:h], e_t[:P, :, h:w], OP.add
                    )
                    w = h
                nc.vector.reduce_sum(s_all[:P, ts], e_t[:P, :, 0:3], axis=AX.X)

                nc.vector.tensor_copy(xsel_all[:P, ts], aux_t[:P, :, 11:12].squeeze())

                nc.vector.tensor_tensor(
                    rhs_all[:P, ts, Q : 2 * Q],
                    iota9[:P, None, :].broadcast_to([P, T, Q]),
                    labq.broadcast_to([P, T, Q]),
                    OP.is_equal,
                )
                nc.vector.tensor_tensor(
                    ar_all[:P, ts, :],
                    iota9[:P, None, :].broadcast_to([P, T, Q]),
                    labr.broadcast_to([P, T, Q]),
                    OP.is_equal,
                )

                # ---- loc path on gpsimd (bf16), strip reduce on vector ----
                df = small.tile([128, T, 4], BF16, tag="df")
                nc.gpsimd.tensor_tensor(
                    df[:P], aux_t[:P, :, 0:4], aux_t[:P, :, 4:8], OP.subtract
                )
                ad = small.tile([128, T, 4], BF16, tag="ad")
                nc.vector.scalar_tensor_tensor(
                    ad[:P], df[:P], -1.0, df[:P], OP.mult, OP.max
                )
                dm = small.tile([128, T, 4], BF16, tag="dm")
                nc.vector.tensor_scalar_min(dm[:P], ad[:P], 1.0)
                r_t = small.tile([128, T, 4], BF16, tag="r")
                nc.gpsimd.tensor_tensor(r_t[:P], ad[:P], dm[:P], OP.subtract)
                q_t = small.tile([128, T, 4], BF16, tag="q")
                nc.gpsimd.tensor_tensor(q_t[:P], dm[:P], dm[:P], OP.mult)
                sl1 = small.tile([128, T, 4], BF16, tag="sl1")
                nc.vector.scalar_tensor_tensor(
                    sl1[:P], q_t[:P], 0.5, r_t[:P], OP.mult, OP.add
                )
                pos = small.tile([128, T], BF16, tag="pos")
                nc.vector.tensor_scalar(pos[:P], lab.squeeze(), 0.0, None, OP.is_gt)
                slm = small.tile([128, T, 4], BF16, tag="slm")
                nc.gpsimd.tensor_tensor(
                    slm[:P], sl1[:P], pos[:P, :, None].broadcast_to([P, T, 4]), OP.mult
                )
                with nc.allow_low_precision("partial sums accumulate in f32 strip"):
                    nc.vector.reduce_sum(strip[:P, i : i + 1], slm[:P], axis=AX.XY)

                if i + PREFETCH < n_tiles:
                    dispatch(i + PREFETCH)

                if (i + 1) % QUAD == 0:
                    phase_b((i + 1) // QUAD - 1)

            # ---- finalize ----
            hc = singles.tile([Q, 2 * Q], F32)
            nc.vector.tensor_copy(hc[:, :], ph_banks[0][:, :])
            for k in range(1, NB):
                nc.vector.tensor_tensor(hc[:, :], hc[:, :], ph_banks[k][:, :], OP.add)
            nc.sync.dma_start(hist[:, :], hc[:, :])
            lacc = singles.tile([128, 1], F32)
            nc.vector.reduce_sum(lacc[:, :], strip[:, :], axis=AX.X)
            nc.sync.dma_start(locs[:, :], lacc[:, :])

    nc.compile()
    return nc


_CACHED = {}


def _get_nc(A, APP, T):
    key = (A, APP, T)
    if key not in _CACHED:
        _CACHED[key] = build_kernel(A, APP, T)
    return _CACHED[key]


def combine_host(hists, locsums, alpha):
    """hists: [ncores, 9, 18]; locsums: [ncores, 128, 1]; alpha: [81]."""
    hw = hists[:, :, 0:Q].sum(axis=0).astype(np.float64)       # [r, q]
    hcnt = hists[:, :, Q : 2 * Q].sum(axis=0).astype(np.float64)
    h = hw.T.ravel()[:C]      # h[9q+r]
    cnt = hcnt.T.ravel()[:C]
    alpha = alpha.astype(np.float64)
    denom = np.clip(alpha * cnt, 1.0, None)
    conf_loss = np.sum(alpha * h / denom)
    num_pos = cnt[1:].sum()
    loc_sum = locsums.astype(np.float64).sum()
    denom_loc = max(num_pos * 4.0, 1.0)
    loc_loss = loc_sum / denom_loc if num_pos > 0 else 0.0
    return np.float32(loc_loss), np.float32(conf_loss)


def kernel(loc_pred, conf_pred, targets, alpha, _trace=False):
    B, A, _ = conf_pred.shape
    assert B == 8 and A == 76725
    AP_ = 76800  # pad to 128*600 so every tile covers all 128 partitions
    nc = _get_nc(AP_, 600, 75)

    conf16 = np.zeros((B, AP_, C), dtype=BF16NP)               # [B, AP_, 81]
    conf16[:, :A] = np.asarray(conf_pred, dtype=BF16NP)
    tgt = np.asarray(targets, dtype=np.float32)
    lab_i = np.full((B, AP_), -1, dtype=np.int32)
    lab_i[:, :A] = tgt[:, :, 4].astype(np.int32)               # [B, AP_]
    labq = lab_i // 9
    labr = lab_i - 9 * labq
    xsel = np.take_along_axis(
        conf16, np.maximum(lab_i, 0)[:, :, None], axis=2
    )[:, :, 0]                                                 # [B, AP_] bf16

    aux = np.zeros((B, AP_, 12), dtype=BF16NP)
    aux[:, :A, 0:4] = loc_pred
    aux[:, :A, 4:8] = tgt[:, :, 0:4]
    aux[:, :, 8] = lab_i
    aux[:, :, 9] = labq
    aux[:, :, 10] = labr
    aux[:, :, 11] = xsel

    in_maps = [
        {
            "conf": np.ascontiguousarray(conf16[b]),
            "aux": np.ascontiguousarray(aux[b]),
        }
        for b in range(B)
    ]
    res = run_bass_kernel_spmd(nc, in_maps, core_ids=list(range(B)), trace=_trace)
    hists = np.stack([r["hist"] for r in res.results])
    locsums = np.stack([r["locs"] for r in res.results])
    out = combine_host(hists, locsums, np.asarray(alpha, dtype=np.float32))
    if _trace:
        return out, res
    return out


# revision 24
# speedup vs baseline: 1.3603x; 1.0017x over previous
"""Focal-loss + smooth-L1 loss kernel for TRN2, SPMD over 8 NeuronCores.

Sharding: data-parallel over the batch axis (B=8 -> one batch row per core).

Host prep (per core), all bf16:
  conf16 [A, 81]  - logits
  aux   [A, 12]   - loc(4), box(4), lab, labq=lab//9, labr=lab-9*labq,
                    xsel=conf[n, max(lab,0)]
Device (per core, anchor n = 600*p + t; tiles of T=75, last tile P=127):
  phase A per tile (pipelined):
    e[:, :, 0:81] = exp(conf)       (scalar engine; e rows padded to 96 with
                                     persistent zero pad cols for the fold)
    s = fold-tree sum_c e (96->48->24->12->6->3->reduce)  (vector, 2x bf16)
    aq -> rhs_all[.., 9:18], ar -> ar_all, xsel -> xsel_all
    smooth-L1 partials on gpsimd, strip reduce on vector
  phase B per quad of tiles (batches ACT table switches):
    lns = ln(s); pt = exp(xsel - lns); w0 = (1-pt)^2*(lns-xsel)
    rhs_all[.., 0:9] = aq * w0
    per-t matmul ph[r, k] += ar_t^T @ rhs_t -> PSUM [9, 18]
Host combine: h[9q+r] = ph[r, q], cnt[9q+r] = ph[r, 9+q]; tiny final math.

All bulk HBM->SBUF transfers go through SWDGE (gpsimd) so descriptors
spread across all 16 SDMA engines (HWDGE pins them to one engine).
"""

import numpy as np
import ml_dtypes

import concourse.bass as bass
import concourse.bacc as bacc
import concourse.mybir as mybir
import concourse.tile as tile
from concourse.bass_utils import run_bass_kernel_spmd

BF16NP = np.dtype(ml_dtypes.bfloat16)

F32 = mybir.dt.float32
BF16 = mybir.dt.bfloat16
I16 = mybir.dt.int16
AF = mybir.ActivationFunctionType
OP = mybir.AluOpType
AX = mybir.AxisListType

C = 81
CP = 96  # padded e-row width (even fold widths: 96/48/24/12/6/3)
Q = 9    # base-9 split: class c = 9*q + r
QUAD = 4  # tiles per phase-B batch


def build_kernel(A, APP, T):
    """A anchors (padded so A == 128*APP -> every tile uses all 128 partitions)."""
    n_tiles = APP // T
    assert A == 128 * APP, (A, APP)

    nc = bacc.Bacc(None, target_bir_lowering=False)
    conf = nc.dram_tensor("conf", [A, C], BF16, kind="ExternalInput")
    aux = nc.dram_tensor("aux", [A, 12], BF16, kind="ExternalInput")
    hist = nc.dram_tensor("hist", [Q, 2 * Q], F32, kind="ExternalOutput")
    locs = nc.dram_tensor("locs", [128, 1], F32, kind="ExternalOutput")

    with tile.TileContext(nc) as tc:
        with (
            tc.tile_pool(name="singles", bufs=1) as singles,
            tc.tile_pool(name="io", bufs=4) as io,
            tc.tile_pool(name="small", bufs=3) as small,
            tc.tile_pool(name="psum", bufs=1, space="PSUM") as psum,
        ):
            # constants / persistent accumulators
            iota_i = singles.tile([128, Q], I16)
            nc.gpsimd.iota(iota_i[:, :], [[1, Q]], channel_multiplier=0)
            iota9 = singles.tile([128, Q], BF16)
            nc.vector.tensor_copy(iota9[:, :], iota_i[:, :])
            strip = singles.tile([128, n_tiles], F32)
            nc.vector.memset(strip[:, :], 0.0)

            NEB = 3
            e_bufs = [
                singles.tile([128, T, CP], BF16, tag=f"e{k}", name=f"e{k}")
                for k in range(NEB)
            ]
            for k in range(NEB):
                nc.vector.memset(e_bufs[k][:, :, C:CP], 0.0)

            s_all = singles.tile([128, APP], F32)
            nc.vector.memset(s_all[:, :], 1.0)
            xsel_all = singles.tile([128, APP], BF16)
            u_all = singles.tile([128, APP], F32)
            lns_all = singles.tile([128, APP], F32)
            pt_all = singles.tile([128, APP], F32)
            pm1_all = singles.tile([128, APP], F32)
            usq_all = singles.tile([128, APP], F32)
            w0_all = singles.tile([128, APP], BF16)
            # comb planes: [.., 0, :] = aq, [.., 1, :] = ar, [.., 2, :] = ar*w0
            comb_all = singles.tile([128, APP, 3, Q], BF16)
            iota18 = singles.tile([128, 2 * Q], BF16)
            nc.vector.tensor_copy(iota18[:, 0:Q], iota_i[:, :])
            nc.vector.tensor_copy(iota18[:, Q : 2 * Q], iota_i[:, :])

            NB = 8  # PSUM banks round-robin to avoid accumulation RAW serialization
            ph_banks = [
                psum.tile([Q, 2 * Q], F32, tag=f"ph{k}", name=f"ph{k}")
                for k in range(NB)
            ]

            def phase_b(qi):
                """Ln/exp batch + matmuls for tiles qi*QUAD .. qi*QUAD+QUAD-1."""
                g0 = qi * QUAD * T
                g1 = min((qi + 1) * QUAD * T, APP)
                W = g1 - g0
                sl = slice(g0, g1)
                nc.scalar.activation(lns_all[:, sl], s_all[:, sl], AF.Ln)
                nc.vector.tensor_tensor(
                    u_all[:, sl], xsel_all[:, sl], lns_all[:, sl], OP.subtract
                )
                nc.scalar.activation(pt_all[:, sl], u_all[:, sl], AF.Exp)
                nc.vector.tensor_scalar_add(pm1_all[:, sl], pt_all[:, sl], -1.0)
                nc.vector.tensor_tensor(
                    usq_all[:, sl], pm1_all[:, sl], pm1_all[:, sl], OP.mult
                )
                nc.vector.scalar_tensor_tensor(
                    w0_all[:, sl], u_all[:, sl], -1.0, usq_all[:, sl],
                    OP.mult, OP.mult,
                )  # w0 = (lns - xsel) * (1-pt)^2
                nc.vector.tensor_tensor(
                    comb_all[:, sl, 2, :],
                    comb_all[:, sl, 1, :],
                    w0_all[:, sl, None].broadcast_to([128, W, Q]),
                    OP.mult,
                )
                for tg in range(g0, g1):
                    nc.tensor.matmul(
                        ph_banks[tg % NB][:, :],
                        comb_all[:, tg, 0, :],
                        comb_all[:, tg, 1:3, :],
                        start=(tg < NB),
                        stop=(tg >= APP - NB),
                    )

            PREFETCH = 3
            pending = {}

            def dispatch(j):
                jt0 = j * T
                conf_t = io.tile([128, T, C], BF16, tag="conf", name="conf_t")
                nc.gpsimd.dma_start(
                    conf_t[:, :, :],
                    bass.AP(
                        tensor=conf[:, :].tensor,
                        offset=jt0 * C,
                        ap=[[APP * C, 128], [C, T], [1, C]],
                    ),
                )
                aux_t = io.tile([128, T, 12], BF16, tag="aux", name="aux_t")
                nc.gpsimd.dma_start(
                    aux_t[:, :, :],
                    bass.AP(
                        tensor=aux[:, :].tensor,
                        offset=jt0 * 12,
                        ap=[[APP * 12, 128], [12, T], [1, 12]],
                    ),
                )
                pending[j] = (conf_t, aux_t)

            for j in range(PREFETCH):
                dispatch(j)

            for i in range(n_tiles):
                t0 = i * T
                P = 128
                ts = slice(t0, t0 + T)

                conf_t, aux_t = pending.pop(i)

                posv = aux_t[:P, :, 8:9]
                labqr = aux_t[:P, :, 9:11]

                # ---- conf path ----
                e_t = e_bufs[i % NEB]
                nc.scalar.activation(e_t[:P, :, 0:C], conf_t[:P], AF.Exp)
                w = CP
                while w > 3:
                    h = w // 2
                    nc.vector.tensor_tensor(
                        e_t[:P, :, 0:h], e_t[:P, :, 0:h], e_t[:P, :, h:w], OP.add
                    )
                    w = h
                nc.vector.reduce_sum(s_all[:P, ts], e_t[:P, :, 0:3], axis=AX.X)

                nc.vector.tensor_copy(xsel_all[:P, ts], aux_t[:P, :, 11:12].squeeze())

                nc.vector.tensor_tensor(
                    comb_all[:P, ts, 0:2, :],
                    iota18[:P, None, :].broadcast_to([P, T, 2 * Q]),
                    labqr[:, :, :, None].broadcast_to([P, T, 2, Q]),
                    OP.is_equal,
                )

                # ---- loc path on gpsimd (bf16), strip reduce on vector ----
                df = small.tile([128, T, 4], BF16, tag="df")
                nc.gpsimd.tensor_tensor(
                    df[:P], aux_t[:P, :, 0:4], aux_t[:P, :, 4:8], OP.subtract
                )
                ad = small.tile([128, T, 4], BF16, tag="ad")
                nc.vector.scalar_tensor_tensor(
                    ad[:P], df[:P], -1.0, df[:P], OP.mult, OP.max
                )
                dm = small.tile([128, T, 4], BF16, tag="dm")
                nc.vector.tensor_scalar_min(dm[:P], ad[:P], 1.0)
                r_t = small.tile([128, T, 4], BF16, tag="r")
                nc.gpsimd.tensor_tensor(r_t[:P], ad[:P], dm[:P], OP.subtract)
                q_t = small.tile([128, T, 4], BF16, tag="q")
                nc.gpsimd.tensor_tensor(q_t[:P], dm[:P], dm[:P], OP.mult)
                sl1 = small.tile([128, T, 4], BF16, tag="sl1")
                nc.vector.scalar_tensor_tensor(
                    sl1[:P], q_t[:P], 0.5, r_t[:P], OP.mult, OP.add
                )
                slm = small.tile([128, T, 4], BF16, tag="slm")
                nc.gpsimd.tensor_tensor(
                    slm[:P], sl1[:P], posv.broadcast_to([P, T, 4]), OP.mult
                )
                with nc.allow_low_precision("partial sums accumulate in f32 strip"):
                    nc.vector.reduce_sum(strip[:P, i : i + 1], slm[:P], axis=AX.XY)

                if i + PREFETCH < n_tiles:
                    dispatch(i + PREFETCH)

                if (i + 1) % QUAD == 0:
                    phase_b((i + 1) // QUAD - 1)

            # ---- finalize ----
            hc = singles.tile([Q, 2 * Q], F32)
            nc.vector.tensor_copy(hc[:, :], ph_banks[0][:, :])
            for k in range(1, NB):
                nc.vector.tensor_tensor(hc[:, :], hc[:, :], ph_banks[k][:, :], OP.add)
            nc.sync.dma_start(hist[:, :], hc[:, :])
            lacc = singles.tile([128, 1], F32)
            nc.vector.reduce_sum(lacc[:, :], strip[:, :], axis=AX.X)
            nc.sync.dma_start(locs[:, :], lacc[:, :])

    nc.compile()
    return nc


_CACHED = {}


def _get_nc(A, APP, T):
    key = (A, APP, T)
    if key not in _CACHED:
        _CACHED[key] = build_kernel(A, APP, T)
    return _CACHED[key]


def combine_host(hists, locsums, alpha):
    """hists: [ncores, 9, 18]; locsums: [ncores, 128, 1]; alpha: [81]."""
    hcnt = hists[:, :, 0:Q].sum(axis=0).astype(np.float64)     # [q, r]
    hw = hists[:, :, Q : 2 * Q].sum(axis=0).astype(np.float64)
    h = hw.ravel()[:C]        # h[9q+r]
    cnt = hcnt.ravel()[:C]
    alpha = alpha.astype(np.float64)
    denom = np.clip(alpha * cnt, 1.0, None)
    conf_loss = np.sum(alpha * h / denom)
    num_pos = cnt[1:].sum()
    loc_sum = locsums.astype(np.float64).sum()
    denom_loc = max(num_pos * 4.0, 1.0)
    loc_loss = loc_sum / denom_loc if num_pos > 0 else 0.0
    return np.float32(loc_loss), np.float32(conf_loss)


def kernel(loc_pred, conf_pred, targets, alpha, _trace=False):
    B, A, _ = conf_pred.shape
    assert B == 8 and A == 76725
    AP_ = 76800  # pad to 128*600 so every tile covers all 128 partitions
    nc = _get_nc(AP_, 600, 75)

    conf16 = np.zeros((B, AP_, C), dtype=BF16NP)               # [B, AP_, 81]
    conf16[:, :A] = np.asarray(conf_pred, dtype=BF16NP)
    tgt = np.asarray(targets, dtype=np.float32)
    lab_i = np.full((B, AP_), -1, dtype=np.int32)
    lab_i[:, :A] = tgt[:, :, 4].astype(np.int32)               # [B, AP_]
    labq = lab_i // 9
    labr = lab_i - 9 * labq
    xsel = np.take_along_axis(
        conf16, np.maximum(lab_i, 0)[:, :, None], axis=2
    )[:, :, 0]                                                 # [B, AP_] bf16

    aux = np.zeros((B, AP_, 12), dtype=BF16NP)
    aux[:, :A, 0:4] = loc_pred
    aux[:, :A, 4:8] = tgt[:, :, 0:4]
    aux[:, :, 8] = (lab_i > 0)
    aux[:, :, 9] = labq
    aux[:, :, 10] = labr
    aux[:, :, 11] = xsel

    in_maps = [
        {
            "conf": np.ascontiguousarray(conf16[b]),
            "aux": np.ascontiguousarray(aux[b]),
        }
        for b in range(B)
    ]
    res = run_bass_kernel_spmd(nc, in_maps, core_ids=list(range(B)), trace=_trace)
    hists = np.stack([r["hist"] for r in res.results])
    locsums = np.stack([r["locs"] for r in res.results])
    out = combine_host(hists, locsums, np.asarray(alpha, dtype=np.float32))
    if _trace:
        return out, res
    return out


# revision 26
# speedup vs baseline: 1.4060x; 1.0336x over previous
"""Focal-loss + smooth-L1 loss kernel for TRN2, SPMD over 8 NeuronCores.

Sharding: data-parallel over the batch axis (B=8 -> one batch row per core).

Host prep (per core), all bf16:
  conf16 [A, 81]  - logits
  aux   [A, 12]   - loc(4), box(4), lab, labq=lab//9, labr=lab-9*labq,
                    xsel=conf[n, max(lab,0)]
Device (per core, anchor n = 600*p + t; tiles of T=75, last tile P=127):
  phase A per tile (pipelined):
    e[:, :, 0:81] = exp(conf)       (scalar engine; e rows padded to 96 with
                                     persistent zero pad cols for the fold)
    s = fold-tree sum_c e (96->48->24->12->6->3->reduce)  (vector, 2x bf16)
    aq -> rhs_all[.., 9:18], ar -> ar_all, xsel -> xsel_all
    smooth-L1 partials on gpsimd, strip reduce on vector
  phase B per quad of tiles (batches ACT table switches):
    lns = ln(s); pt = exp(xsel - lns); w0 = (1-pt)^2*(lns-xsel)
    rhs_all[.., 0:9] = aq * w0
    per-t matmul ph[r, k] += ar_t^T @ rhs_t -> PSUM [9, 18]
Host combine: h[9q+r] = ph[r, q], cnt[9q+r] = ph[r, 9+q]; tiny final math.

All bulk HBM->SBUF transfers go through SWDGE (gpsimd) so descriptors
spread across all 16 SDMA engines (HWDGE pins them to one engine).
"""

import numpy as np
import ml_dtypes

import concourse.bass as bass
import concourse.bacc as bacc
import concourse.mybir as mybir
import concourse.tile as tile
from concourse.bass_utils import run_bass_kernel_spmd

BF16NP = np.dtype(ml_dtypes.bfloat16)

F32 = mybir.dt.float32
BF16 = mybir.dt.bfloat16
I16 = mybir.dt.int16
AF = mybir.ActivationFunctionType
OP = mybir.AluOpType
AX = mybir.AxisListType

C = 81
CP = 96  # padded e-row width (even fold widths: 96/48/24/12/6/3)
Q = 9    # base-9 split: class c = 9*q + r
QUAD = 2  # tiles per phase-B batch


def build_kernel(A, APP, T):
    """A anchors (padded so A == 128*APP -> every tile uses all 128 partitions)."""
    n_tiles = APP // T
    assert A == 128 * APP, (A, APP)

    nc = bacc.Bacc(None, target_bir_lowering=False)
    conf = nc.dram_tensor("conf", [A, C], BF16, kind="ExternalInput")
    aux = nc.dram_tensor("aux", [A, 12], BF16, kind="ExternalInput")
    hist = nc.dram_tensor("hist", [Q, 2 * Q], F32, kind="ExternalOutput")
    locs = nc.dram_tensor("locs", [128, 1], F32, kind="ExternalOutput")

    with tile.TileContext(nc) as tc:
        with (
            tc.tile_pool(name="singles", bufs=1) as singles,
            tc.tile_pool(name="io", bufs=4) as io,
            tc.tile_pool(name="small", bufs=3) as small,
            tc.tile_pool(name="psum", bufs=1, space="PSUM") as psum,
        ):
            # constants / persistent accumulators
            iota_i = singles.tile([128, Q], I16)
            nc.gpsimd.iota(iota_i[:, :], [[1, Q]], channel_multiplier=0)
            iota9 = singles.tile([128, Q], BF16)
            nc.vector.tensor_copy(iota9[:, :], iota_i[:, :])
            strip = singles.tile([128, n_tiles], F32)
            nc.vector.memset(strip[:, :], 0.0)
            ones1 = singles.tile([128, 1], BF16)
            nc.vector.memset(ones1[:, :], 1.0)

            NEB = 3
            e_bufs = [
                singles.tile([128, T, CP], BF16, tag=f"e{k}", name=f"e{k}")
                for k in range(NEB)
            ]
            for k in range(NEB):
                nc.vector.memset(e_bufs[k][:, :, C:CP], 0.0)

            s_all = singles.tile([128, APP], F32)
            nc.vector.memset(s_all[:, :], 1.0)
            xsel_all = singles.tile([128, APP], BF16)
            u_all = singles.tile([128, APP], F32)
            lns_all = singles.tile([128, APP], F32)
            pt_all = singles.tile([128, APP], F32)
            pm1_all = singles.tile([128, APP], F32)
            usq_all = singles.tile([128, APP], F32)
            w0_all = singles.tile([128, APP], BF16)
            # comb planes: [.., 0, :] = aq, [.., 1, :] = ar, [.., 2, :] = ar*w0
            comb_all = singles.tile([128, APP, 3, Q], BF16)
            iota18 = singles.tile([128, 2 * Q], BF16)
            nc.vector.tensor_copy(iota18[:, 0:Q], iota_i[:, :])
            nc.vector.tensor_copy(iota18[:, Q : 2 * Q], iota_i[:, :])

            NB = 8  # PSUM banks round-robin to avoid accumulation RAW serialization
            ph_banks = [
                psum.tile([Q, 2 * Q], F32, tag=f"ph{k}", name=f"ph{k}")
                for k in range(NB)
            ]

            def phase_b(qi):
                """Ln/exp batch + matmuls for tiles qi*QUAD .. qi*QUAD+QUAD-1."""
                g0 = qi * QUAD * T
                g1 = min((qi + 1) * QUAD * T, APP)
                W = g1 - g0
                sl = slice(g0, g1)
                nc.scalar.activation(lns_all[:, sl], s_all[:, sl], AF.Ln)
                nc.vector.tensor_tensor(
                    u_all[:, sl], xsel_all[:, sl], lns_all[:, sl], OP.subtract
                )
                nc.scalar.activation(pt_all[:, sl], u_all[:, sl], AF.Exp)
                nc.vector.tensor_scalar_add(pm1_all[:, sl], pt_all[:, sl], -1.0)
                nc.vector.tensor_tensor(
                    usq_all[:, sl], pm1_all[:, sl], pm1_all[:, sl], OP.mult
                )
                nc.vector.scalar_tensor_tensor(
                    w0_all[:, sl], u_all[:, sl], -1.0, usq_all[:, sl],
                    OP.mult, OP.mult,
                )  # w0 = (lns - xsel) * (1-pt)^2
                nc.vector.tensor_tensor(
                    comb_all[:, sl, 2, :],
                    comb_all[:, sl, 1, :],
                    w0_all[:, sl, None].broadcast_to([128, W, Q]),
                    OP.mult,
                )
                for tg in range(g0, g1):
                    nc.tensor.matmul(
                        ph_banks[tg % NB][:, :],
                        comb_all[:, tg, 0, :],
                        comb_all[:, tg, 1:3, :],
                        start=(tg < NB),
                        stop=(tg >= APP - NB),
                    )

            PREFETCH = 3
            pending = {}

            def dispatch(j):
                jt0 = j * T
                conf_t = io.tile([128, T, C], BF16, tag="conf", name="conf_t")
                nc.gpsimd.dma_start(
                    conf_t[:, :, :],
                    bass.AP(
                        tensor=conf[:, :].tensor,
                        offset=jt0 * C,
                        ap=[[APP * C, 128], [C, T], [1, C]],
                    ),
                )
                aux_t = io.tile([128, T, 12], BF16, tag="aux", name="aux_t")
                nc.gpsimd.dma_start(
                    aux_t[:, :, :],
                    bass.AP(
                        tensor=aux[:, :].tensor,
                        offset=jt0 * 12,
                        ap=[[APP * 12, 128], [12, T], [1, 12]],
                    ),
                )
                pending[j] = (conf_t, aux_t)

            for j in range(PREFETCH):
                dispatch(j)

            for i in range(n_tiles):
                t0 = i * T
                P = 128
                ts = slice(t0, t0 + T)

                conf_t, aux_t = pending.pop(i)

                posv = aux_t[:P, :, 8:9]
                labqr = aux_t[:P, :, 9:11]

                # ---- conf path ----
                e_t = e_bufs[i % NEB]
                nc.scalar.activation(e_t[:P, :, 0:C], conf_t[:P], AF.Exp)
                w = CP
                while w > 3:
                    h = w // 2
                    nc.vector.tensor_tensor(
                        e_t[:P, :, 0:h], e_t[:P, :, 0:h], e_t[:P, :, h:w], OP.add
                    )
                    w = h
                nc.vector.reduce_sum(s_all[:P, ts], e_t[:P, :, 0:3], axis=AX.X)

                nc.vector.tensor_copy(xsel_all[:P, ts], aux_t[:P, :, 11:12].squeeze())

                nc.vector.tensor_tensor(
                    comb_all[:P, ts, 0:2, :],
                    iota18[:P, None, :].broadcast_to([P, T, 2 * Q]),
                    labqr[:, :, :, None].broadcast_to([P, T, 2, Q]),
                    OP.is_equal,
                )

                # ---- loc path on gpsimd (bf16), strip reduce on vector ----
                df = small.tile([128, T, 4], BF16, tag="df")
                nc.gpsimd.tensor_tensor(
                    df[:P], aux_t[:P, :, 0:4], aux_t[:P, :, 4:8], OP.subtract
                )
                ad = small.tile([128, T, 4], BF16, tag="ad")
                nc.scalar.activation(ad[:P], df[:P], AF.Abs)
                dm = small.tile([128, T, 4], BF16, tag="dm")
                nc.vector.tensor_scalar_min(dm[:P], ad[:P], 1.0)
                r_t = small.tile([128, T, 4], BF16, tag="r")
                nc.gpsimd.tensor_tensor(r_t[:P], ad[:P], dm[:P], OP.subtract)
                q_t = small.tile([128, T, 4], BF16, tag="q")
                nc.gpsimd.tensor_tensor(q_t[:P], dm[:P], dm[:P], OP.mult)
                sl1 = small.tile([128, T, 4], BF16, tag="sl1")
                nc.vector.scalar_tensor_tensor(
                    sl1[:P], q_t[:P], 0.5, r_t[:P], OP.mult, OP.add
                )
                slm = small.tile([128, T, 4], BF16, tag="slm")
                nc.gpsimd.tensor_tensor(
                    slm[:P], sl1[:P], posv.broadcast_to([P, T, 4]), OP.mult
                )
                with nc.allow_low_precision("partial sums accumulate in f32 strip"):
                    nc.vector.reduce_sum(strip[:P, i : i + 1], slm[:P], axis=AX.XY)

                if i + PREFETCH < n_tiles:
                    dispatch(i + PREFETCH)

                if (i + 1) % QUAD == 0:
                    phase_b((i + 1) // QUAD - 1)

            # ---- finalize ----
            hc = singles.tile([Q, 2 * Q], F32)
            nc.vector.tensor_copy(hc[:, :], ph_banks[0][:, :])
            for k in range(1, NB):
                nc.vector.tensor_tensor(hc[:, :], hc[:, :], ph_banks[k][:, :], OP.add)
            nc.sync.dma_start(hist[:, :], hc[:, :])
            lacc = singles.tile([128, 1], F32)
            nc.vector.reduce_sum(lacc[:, :], strip[:, :], axis=AX.X)
            nc.sync.dma_start(locs[:, :], lacc[:, :])

    nc.compile()
    return nc


_CACHED = {}


def _get_nc(A, APP, T):
    key = (A, APP, T)
    if key not in _CACHED:
        _CACHED[key] = build_kernel(A, APP, T)
    return _CACHED[key]


def combine_host(hists, locsums, alpha):
    """hists: [ncores, 9, 18]; locsums: [ncores, 128, 1]; alpha: [81]."""
    hcnt = hists[:, :, 0:Q].sum(axis=0).astype(np.float64)     # [q, r]
    hw = hists[:, :, Q : 2 * Q].sum(axis=0).astype(np.float64)
    h = hw.ravel()[:C]        # h[9q+r]
    cnt = hcnt.ravel()[:C]
    alpha = alpha.astype(np.float64)
    denom = np.clip(alpha * cnt, 1.0, None)
    conf_loss = np.sum(alpha * h / denom)
    num_pos = cnt[1:].sum()
    loc_sum = locsums.astype(np.float64).sum()
    denom_loc = max(num_pos * 4.0, 1.0)
    loc_loss = loc_sum / denom_loc if num_pos > 0 else 0.0
    return np.float32(loc_loss), np.float32(conf_loss)


def kernel(loc_pred, conf_pred, targets, alpha, _trace=False):
    B, A, _ = conf_pred.shape
    assert B == 8 and A == 76725
    AP_ = 76800  # pad to 128*600 so every tile covers all 128 partitions
    nc = _get_nc(AP_, 600, 75)

    conf16 = np.zeros((B, AP_, C), dtype=BF16NP)               # [B, AP_, 81]
    conf16[:, :A] = np.asarray(conf_pred, dtype=BF16NP)
    tgt = np.asarray(targets, dtype=np.float32)
    lab_i = np.full((B, AP_), -1, dtype=np.int32)
    lab_i[:, :A] = tgt[:, :, 4].astype(np.int32)               # [B, AP_]
    labq = lab_i // 9
    labr = lab_i - 9 * labq
    xsel = np.take_along_axis(
        conf16, np.maximum(lab_i, 0)[:, :, None], axis=2
    )[:, :, 0]                                                 # [B, AP_] bf16

    aux = np.zeros((B, AP_, 12), dtype=BF16NP)
    aux[:, :A, 0:4] = loc_pred
    aux[:, :A, 4:8] = tgt[:, :, 0:4]
    aux[:, :, 8] = (lab_i > 0)
    aux[:, :, 9] = labq
    aux[:, :, 10] = labr
    aux[:, :, 11] = xsel

    in_maps = [
        {
            "conf": np.ascontiguousarray(conf16[b]),
            "aux": np.ascontiguousarray(aux[b]),
        }
        for b in range(B)
    ]
    res = run_bass_kernel_spmd(nc, in_maps, core_ids=list(range(B)), trace=_trace)
    hists = np.stack([r["hist"] for r in res.results])
    locsums = np.stack([r["locs"] for r in res.results])
    out = combine_host(hists, locsums, np.asarray(alpha, dtype=np.float32))
    if _trace:
        return out, res
    return out


# revision 27
# speedup vs baseline: 1.4682x; 1.0442x over previous
"""Focal-loss + smooth-L1 loss kernel for TRN2, SPMD over 8 NeuronCores.

Sharding: data-parallel over the batch axis (B=8 -> one batch row per core).

Host prep (per core), all bf16:
  conf16 [A, 81]  - logits
  aux   [A, 12]   - loc(4), box(4), lab, labq=lab//9, labr=lab-9*labq,
                    xsel=conf[n, max(lab,0)]
Device (per core, anchor n = 600*p + t; tiles of T=75, last tile P=127):
  phase A per tile (pipelined):
    e[:, :, 0:81] = exp(conf)       (scalar engine; e rows padded to 96 with
                                     persistent zero pad cols for the fold)
    s = fold-tree sum_c e (96->48->24->12->6->3->reduce)  (vector, 2x bf16)
    aq -> rhs_all[.., 9:18], ar -> ar_all, xsel -> xsel_all
    smooth-L1 partials on gpsimd, strip reduce on vector
  phase B per quad of tiles (batches ACT table switches):
    lns = ln(s); pt = exp(xsel - lns); w0 = (1-pt)^2*(lns-xsel)
    rhs_all[.., 0:9] = aq * w0
    per-t matmul ph[r, k] += ar_t^T @ rhs_t -> PSUM [9, 18]
Host combine: h[9q+r] = ph[r, q], cnt[9q+r] = ph[r, 9+q]; tiny final math.

All bulk HBM->SBUF transfers go through SWDGE (gpsimd) so descriptors
spread across all 16 SDMA engines (HWDGE pins them to one engine).
"""

import numpy as np
import ml_dtypes

import concourse.bass as bass
import concourse.bacc as bacc
import concourse.mybir as mybir
import concourse.tile as tile
from concourse.bass_utils import run_bass_kernel_spmd

BF16NP = np.dtype(ml_dtypes.bfloat16)

F32 = mybir.dt.float32
BF16 = mybir.dt.bfloat16
I16 = mybir.dt.int16
AF = mybir.ActivationFunctionType
OP = mybir.AluOpType
AX = mybir.AxisListType

C = 81
CP = 96  # padded e-row width (even fold widths: 96/48/24/12/6/3)
Q = 9    # base-9 split: class c = 9*q + r
QUAD = 2  # tiles per phase-B batch


def build_kernel(A, APP, T):
    """A anchors (padded so A == 128*APP -> every tile uses all 128 partitions)."""
    n_tiles = APP // T
    assert A == 128 * APP, (A, APP)

    nc = bacc.Bacc(None, target_bir_lowering=False)
    conf = nc.dram_tensor("conf", [A, C], BF16, kind="ExternalInput")
    aux = nc.dram_tensor("aux", [A, 12], BF16, kind="ExternalInput")
    outt = nc.dram_tensor("outt", [128, 2 * Q + 1], F32, kind="ExternalOutput")

    with tile.TileContext(nc) as tc:
        with (
            tc.tile_pool(name="singles", bufs=1) as singles,
            tc.tile_pool(name="io", bufs=4) as io,
            tc.tile_pool(name="small", bufs=3) as small,
            tc.tile_pool(name="psum", bufs=1, space="PSUM") as psum,
        ):
            # constants / persistent accumulators
            iota_i = singles.tile([128, Q], I16)
            nc.gpsimd.iota(iota_i[:, :], [[1, Q]], channel_multiplier=0)
            iota9 = singles.tile([128, Q], BF16)
            nc.vector.tensor_copy(iota9[:, :], iota_i[:, :])
            strip = singles.tile([128, n_tiles], F32)
            nc.vector.memset(strip[:, :], 0.0)
            ones1 = singles.tile([128, 1], BF16)
            nc.vector.memset(ones1[:, :], 1.0)

            NEB = 3
            e_bufs = [
                singles.tile([128, T, CP], BF16, tag=f"e{k}", name=f"e{k}")
                for k in range(NEB)
            ]
            for k in range(NEB):
                nc.vector.memset(e_bufs[k][:, :, C:CP], 0.0)

            s_all = singles.tile([128, APP], F32)
            nc.vector.memset(s_all[:, :], 1.0)
            xsel_all = singles.tile([128, APP], BF16)
            u_all = singles.tile([128, APP], F32)
            lns_all = singles.tile([128, APP], F32)
            pt_all = singles.tile([128, APP], F32)
            pm1_all = singles.tile([128, APP], F32)
            usq_all = singles.tile([128, APP], F32)
            w0_all = singles.tile([128, APP], BF16)
            # comb planes: [.., 0, :] = aq, [.., 1, :] = ar, [.., 2, :] = ar*w0
            comb_all = singles.tile([128, APP, 3, Q], BF16)
            iota18 = singles.tile([128, 2 * Q], BF16)
            nc.vector.tensor_copy(iota18[:, 0:Q], iota_i[:, :])
            nc.vector.tensor_copy(iota18[:, Q : 2 * Q], iota_i[:, :])

            NB = 8  # PSUM banks round-robin to avoid accumulation RAW serialization
            ph_banks = [
                psum.tile([Q, 2 * Q], F32, tag=f"ph{k}", name=f"ph{k}")
                for k in range(NB)
            ]

            def phase_b(i0, i1):
                """Ln/exp batch + matmuls for tiles i0..i1-1."""
                g0 = i0 * T
                g1 = i1 * T
                W = g1 - g0
                sl = slice(g0, g1)
                nc.scalar.activation(lns_all[:, sl], s_all[:, sl], AF.Ln)
                nc.vector.tensor_tensor(
                    u_all[:, sl], xsel_all[:, sl], lns_all[:, sl], OP.subtract
                )
                nc.scalar.activation(pt_all[:, sl], u_all[:, sl], AF.Exp)
                nc.vector.tensor_scalar_add(pm1_all[:, sl], pt_all[:, sl], -1.0)
                nc.vector.tensor_tensor(
                    usq_all[:, sl], pm1_all[:, sl], pm1_all[:, sl], OP.mult
                )
                nc.vector.scalar_tensor_tensor(
                    w0_all[:, sl], u_all[:, sl], -1.0, usq_all[:, sl],
                    OP.mult, OP.mult,
                )  # w0 = (lns - xsel) * (1-pt)^2
                nc.vector.tensor_tensor(
                    comb_all[:, sl, 2, :],
                    comb_all[:, sl, 1, :],
                    w0_all[:, sl, None].broadcast_to([128, W, Q]),
                    OP.mult,
                )
                for tg in range(g0, g1):
                    nc.tensor.matmul(
                        ph_banks[tg % NB][:, :],
                        comb_all[:, tg, 0, :],
                        comb_all[:, tg, 1:3, :],
                        start=(tg < NB),
                        stop=(tg >= APP - NB),
                    )

            # phase-B groups: pairs early (amortize ACT table loads),
            # singles at the end (short critical-path tail)
            PB_AFTER = {1: 0, 3: 2, 5: 4, 6: 6, 7: 7}
            PREFETCH = 3
            pending = {}

            def dispatch(j):
                jt0 = j * T
                conf_t = io.tile([128, T, C], BF16, tag="conf", name="conf_t")
                nc.gpsimd.dma_start(
                    conf_t[:, :, :],
                    bass.AP(
                        tensor=conf[:, :].tensor,
                        offset=jt0 * C,
                        ap=[[APP * C, 128], [C, T], [1, C]],
                    ),
                )
                aux_t = io.tile([128, T, 12], BF16, tag="aux", name="aux_t")
                nc.gpsimd.dma_start(
                    aux_t[:, :, :],
                    bass.AP(
                        tensor=aux[:, :].tensor,
                        offset=jt0 * 12,
                        ap=[[APP * 12, 128], [12, T], [1, 12]],
                    ),
                )
                pending[j] = (conf_t, aux_t)

            for j in range(PREFETCH):
                dispatch(j)

            for i in range(n_tiles):
                t0 = i * T
                P = 128
                ts = slice(t0, t0 + T)

                conf_t, aux_t = pending.pop(i)

                posv = aux_t[:P, :, 8:9]
                labqr = aux_t[:P, :, 9:11]

                # ---- conf path ----
                e_t = e_bufs[i % NEB]
                nc.scalar.activation(e_t[:P, :, 0:C], conf_t[:P], AF.Exp)
                w = CP
                while w > 3:
                    h = w // 2
                    nc.vector.tensor_tensor(
                        e_t[:P, :, 0:h], e_t[:P, :, 0:h], e_t[:P, :, h:w], OP.add
                    )
                    w = h
                nc.vector.reduce_sum(s_all[:P, ts], e_t[:P, :, 0:3], axis=AX.X)

                nc.vector.tensor_copy(xsel_all[:P, ts], aux_t[:P, :, 11:12].squeeze())

                nc.vector.tensor_tensor(
                    comb_all[:P, ts, 0:2, :],
                    iota18[:P, None, :].broadcast_to([P, T, 2 * Q]),
                    labqr[:, :, :, None].broadcast_to([P, T, 2, Q]),
                    OP.is_equal,
                )

                # ---- loc path on gpsimd (bf16), strip reduce on vector ----
                df = small.tile([128, T, 4], BF16, tag="df")
                nc.gpsimd.tensor_tensor(
                    df[:P], aux_t[:P, :, 0:4], aux_t[:P, :, 4:8], OP.subtract
                )
                ad = small.tile([128, T, 4], BF16, tag="ad")
                nc.scalar.activation(ad[:P], df[:P], AF.Abs)
                dm = small.tile([128, T, 4], BF16, tag="dm")
                nc.vector.tensor_scalar_min(dm[:P], ad[:P], 1.0)
                r_t = small.tile([128, T, 4], BF16, tag="r")
                nc.gpsimd.tensor_tensor(r_t[:P], ad[:P], dm[:P], OP.subtract)
                q_t = small.tile([128, T, 4], BF16, tag="q")
                nc.gpsimd.tensor_tensor(q_t[:P], dm[:P], dm[:P], OP.mult)
                sl1 = small.tile([128, T, 4], BF16, tag="sl1")
                nc.vector.scalar_tensor_tensor(
                    sl1[:P], q_t[:P], 0.5, r_t[:P], OP.mult, OP.add
                )
                slm = small.tile([128, T, 4], BF16, tag="slm")
                nc.gpsimd.tensor_tensor(
                    slm[:P], sl1[:P], posv.broadcast_to([P, T, 4]), OP.mult
                )
                with nc.allow_low_precision("partial sums accumulate in f32 strip"):
                    nc.vector.reduce_sum(strip[:P, i : i + 1], slm[:P], axis=AX.XY)

                if i + PREFETCH < n_tiles:
                    dispatch(i + PREFETCH)

                if i in PB_AFTER:
                    phase_b(PB_AFTER[i], i + 1)

            # ---- finalize: combined [128, 19] output, single DMA ----
            fin = singles.tile([128, 2 * Q + 1], F32)
            nc.vector.reduce_sum(fin[:, 2 * Q : 2 * Q + 1], strip[:, :], axis=AX.X)
            nc.vector.tensor_copy(fin[0:Q, 0 : 2 * Q], ph_banks[0][:, :])
            for k in range(1, NB):
                nc.vector.tensor_tensor(
                    fin[0:Q, 0 : 2 * Q], fin[0:Q, 0 : 2 * Q], ph_banks[k][:, :], OP.add
                )
            nc.gpsimd.dma_start(outt[:, :], fin[:, :])

    nc.compile()
    return nc


_CACHED = {}


def _get_nc(A, APP, T):
    key = (A, APP, T)
    if key not in _CACHED:
        _CACHED[key] = build_kernel(A, APP, T)
    return _CACHED[key]


def combine_host(hists, locsums, alpha):
    """hists: [ncores, 9, 18]; locsums: [ncores, 128, 1]; alpha: [81]."""
    hcnt = hists[:, :, 0:Q].sum(axis=0).astype(np.float64)     # [q, r]
    hw = hists[:, :, Q : 2 * Q].sum(axis=0).astype(np.float64)
    h = hw.ravel()[:C]        # h[9q+r]
    cnt = hcnt.ravel()[:C]
    alpha = alpha.astype(np.float64)
    denom = np.clip(alpha * cnt, 1.0, None)
    conf_loss = np.sum(alpha * h / denom)
    num_pos = cnt[1:].sum()
    loc_sum = locsums.astype(np.float64).sum()
    denom_loc = max(num_pos * 4.0, 1.0)
    loc_loss = loc_sum / denom_loc if num_pos > 0 else 0.0
    return np.float32(loc_loss), np.float32(conf_loss)


def kernel(loc_pred, conf_pred, targets, alpha, _trace=False):
    B, A, _ = conf_pred.shape
    assert B == 8 and A == 76725
    AP_ = 76800  # pad to 128*600 so every tile covers all 128 partitions
    nc = _get_nc(AP_, 600, 75)

    conf16 = np.zeros((B, AP_, C), dtype=BF16NP)               # [B, AP_, 81]
    conf16[:, :A] = np.asarray(conf_pred, dtype=BF16NP)
    tgt = np.asarray(targets, dtype=np.float32)
    lab_i = np.full((B, AP_), -1, dtype=np.int32)
    lab_i[:, :A] = tgt[:, :, 4].astype(np.int32)               # [B, AP_]
    labq = lab_i // 9
    labr = lab_i - 9 * labq
    xsel = np.take_along_axis(
        conf16, np.maximum(lab_i, 0)[:, :, None], axis=2
    )[:, :, 0]                                                 # [B, AP_] bf16

    aux = np.zeros((B, AP_, 12), dtype=BF16NP)
    aux[:, :A, 0:4] = loc_pred
    aux[:, :A, 4:8] = tgt[:, :, 0:4]
    aux[:, :, 8] = (lab_i > 0)
    aux[:, :, 9] = labq
    aux[:, :, 10] = labr
    aux[:, :, 11] = xsel

    in_maps = [
        {
            "conf": np.ascontiguousarray(conf16[b]),
            "aux": np.ascontiguousarray(aux[b]),
        }
        for b in range(B)
    ]
    res = run_bass_kernel_spmd(nc, in_maps, core_ids=list(range(B)), trace=_trace)
    hists = np.stack([r["outt"][0:Q, 0 : 2 * Q] for r in res.results])
    locsums = np.stack([r["outt"][:, 2 * Q] for r in res.results])
    out = combine_host(hists, locsums, np.asarray(alpha, dtype=np.float32))
    if _trace:
        return out, res
    return out


# revision 29
# speedup vs baseline: 1.4916x; 1.0160x over previous
"""Focal-loss + smooth-L1 loss kernel for TRN2, SPMD over 8 NeuronCores.

Sharding: data-parallel over the batch axis (B=8 -> one batch row per core).

Host prep (per core), all bf16:
  conf16 [A, 81]  - logits
  aux   [A, 12]   - loc(4), box(4), lab, labq=lab//9, labr=lab-9*labq,
                    xsel=conf[n, max(lab,0)]
Device (per core, anchor n = 600*p + t; tiles of T=75, last tile P=127):
  phase A per tile (pipelined):
    e[:, :, 0:81] = exp(conf)       (scalar engine; e rows padded to 96 with
                                     persistent zero pad cols for the fold)
    s = fold-tree sum_c e (96->48->24->12->6->3->reduce)  (vector, 2x bf16)
    aq -> rhs_all[.., 9:18], ar -> ar_all, xsel -> xsel_all
    smooth-L1 partials on gpsimd, strip reduce on vector
  phase B per quad of tiles (batches ACT table switches):
    lns = ln(s); pt = exp(xsel - lns); w0 = (1-pt)^2*(lns-xsel)
    rhs_all[.., 0:9] = aq * w0
    per-t matmul ph[r, k] += ar_t^T @ rhs_t -> PSUM [9, 18]
Host combine: h[9q+r] = ph[r, q], cnt[9q+r] = ph[r, 9+q]; tiny final math.

All bulk HBM->SBUF transfers go through SWDGE (gpsimd) so descriptors
spread across all 16 SDMA engines (HWDGE pins them to one engine).
"""

import numpy as np
import ml_dtypes

import concourse.bass as bass
import concourse.bacc as bacc
import concourse.mybir as mybir
import concourse.tile as tile
from concourse.bass_utils import run_bass_kernel_spmd

BF16NP = np.dtype(ml_dtypes.bfloat16)

F32 = mybir.dt.float32
BF16 = mybir.dt.bfloat16
I16 = mybir.dt.int16
AF = mybir.ActivationFunctionType
OP = mybir.AluOpType
AX = mybir.AxisListType

C = 81
CP = 96  # padded e-row width (even fold widths: 96/48/24/12/6/3)
Q = 9    # base-9 split: class c = 9*q + r
QUAD = 2  # tiles per phase-B batch


def build_kernel(A, APP, T):
    """A anchors (padded so A == 128*APP -> every tile uses all 128 partitions)."""
    n_tiles = APP // T
    assert A == 128 * APP, (A, APP)

    nc = bacc.Bacc(None, target_bir_lowering=False)
    conf = nc.dram_tensor("conf", [A, C], BF16, kind="ExternalInput")
    aux = nc.dram_tensor("aux", [A, 12], BF16, kind="ExternalInput")
    outt = nc.dram_tensor("outt", [128, 2 * Q + 1], F32, kind="ExternalOutput")

    with tile.TileContext(nc) as tc:
        with (
            tc.tile_pool(name="singles", bufs=1) as singles,
            tc.tile_pool(name="io", bufs=4) as io,
            tc.tile_pool(name="small", bufs=3) as small,
            tc.tile_pool(name="psum", bufs=1, space="PSUM") as psum,
        ):
            # constants / persistent accumulators
            iota_i = singles.tile([128, Q], I16)
            nc.gpsimd.iota(iota_i[:, :], [[1, Q]], channel_multiplier=0)
            iota9 = singles.tile([128, Q], BF16)
            nc.vector.tensor_copy(iota9[:, :], iota_i[:, :])
            strip = singles.tile([128, n_tiles], F32)
            nc.vector.memset(strip[:, :], 0.0)
            ones1 = singles.tile([128, 1], BF16)
            nc.vector.memset(ones1[:, :], 1.0)

            NEB = 3
            e_bufs = [
                singles.tile([128, T, CP], BF16, tag=f"e{k}", name=f"e{k}")
                for k in range(NEB)
            ]
            for k in range(NEB):
                nc.vector.memset(e_bufs[k][:, :, C:CP], 0.0)

            s_all = singles.tile([128, APP], F32)
            nc.vector.memset(s_all[:, :], 1.0)
            xsel_all = singles.tile([128, APP], BF16)
            u_all = singles.tile([128, APP], F32)
            lns_all = singles.tile([128, APP], F32)
            pt_all = singles.tile([128, APP], F32)
            pm1_all = singles.tile([128, APP], F32)
            usq_all = singles.tile([128, APP], F32)
            w0_all = singles.tile([128, APP], BF16)
            # comb planes: [.., 0, :] = aq, [.., 1, :] = ar, [.., 2, :] = ar*w0
            comb_all = singles.tile([128, APP, 3, Q], BF16)
            iota18 = singles.tile([128, 2 * Q], BF16)
            nc.vector.tensor_copy(iota18[:, 0:Q], iota_i[:, :])
            nc.vector.tensor_copy(iota18[:, Q : 2 * Q], iota_i[:, :])

            NB = 8  # PSUM banks round-robin to avoid accumulation RAW serialization
            ph_banks = [
                psum.tile([Q, 2 * Q], F32, tag=f"ph{k}", name=f"ph{k}")
                for k in range(NB)
            ]

            def phase_b(i0, i1):
                """Ln/exp batch + matmuls for tiles i0..i1-1."""
                g0 = i0 * T
                g1 = i1 * T
                W = g1 - g0
                sl = slice(g0, g1)
                nc.scalar.activation(lns_all[:, sl], s_all[:, sl], AF.Ln)
                nc.vector.tensor_tensor(
                    u_all[:, sl], xsel_all[:, sl], lns_all[:, sl], OP.subtract
                )
                nc.scalar.activation(pt_all[:, sl], u_all[:, sl], AF.Exp)
                nc.vector.tensor_scalar_add(pm1_all[:, sl], pt_all[:, sl], -1.0)
                nc.vector.tensor_tensor(
                    usq_all[:, sl], pm1_all[:, sl], pm1_all[:, sl], OP.mult
                )
                nc.vector.scalar_tensor_tensor(
                    w0_all[:, sl], u_all[:, sl], -1.0, usq_all[:, sl],
                    OP.mult, OP.mult,
                )  # w0 = (lns - xsel) * (1-pt)^2
                nc.vector.tensor_tensor(
                    comb_all[:, sl, 2, :],
                    comb_all[:, sl, 1, :],
                    w0_all[:, sl, None].broadcast_to([128, W, Q]),
                    OP.mult,
                )
                for tg in range(g0, g1):
                    nc.tensor.matmul(
                        ph_banks[tg % NB][:, :],
                        comb_all[:, tg, 0, :],
                        comb_all[:, tg, 1:3, :],
                        start=(tg < NB),
                        stop=(tg >= APP - NB),
                    )

            # phase-B groups: pairs early (amortize ACT table loads),
            # singles at the end (short critical-path tail)
            PB_AFTER = {1: 0, 3: 2, 5: 4, 6: 6, 7: 7}
            PREFETCH = 3
            pending = {}

            def dispatch(j):
                jt0 = j * T
                conf_t = io.tile([128, T, C], BF16, tag="conf", name="conf_t")
                nc.gpsimd.dma_start(
                    conf_t[:, :, :],
                    bass.AP(
                        tensor=conf[:, :].tensor,
                        offset=jt0 * C,
                        ap=[[APP * C, 128], [C, T], [1, C]],
                    ),
                )
                aux_t = io.tile([128, T, 12], BF16, tag="aux", name="aux_t")
                nc.gpsimd.dma_start(
                    aux_t[:, :, :],
                    bass.AP(
                        tensor=aux[:, :].tensor,
                        offset=jt0 * 12,
                        ap=[[APP * 12, 128], [12, T], [1, 12]],
                    ),
                )
                pending[j] = (conf_t, aux_t)

            for j in range(PREFETCH):
                dispatch(j)

            for i in range(n_tiles):
                t0 = i * T
                P = 128
                ts = slice(t0, t0 + T)

                conf_t, aux_t = pending.pop(i)

                posv = aux_t[:P, :, 8:9]
                labqr = aux_t[:P, :, 9:11]

                # ---- conf path ----
                e_t = e_bufs[i % NEB]
                nc.scalar.activation(e_t[:P, :, 0:C], conf_t[:P], AF.Exp)
                w = CP
                while w > 3:
                    h = w // 2
                    nc.vector.tensor_tensor(
                        e_t[:P, :, 0:h], e_t[:P, :, 0:h], e_t[:P, :, h:w], OP.add
                    )
                    w = h
                nc.vector.reduce_sum(s_all[:P, ts], e_t[:P, :, 0:3], axis=AX.X)

                nc.vector.tensor_copy(xsel_all[:P, ts], aux_t[:P, :, 11:12].squeeze())

                nc.vector.tensor_tensor(
                    comb_all[:P, ts, 0:2, :],
                    iota18[:P, None, :].broadcast_to([P, T, 2 * Q]),
                    labqr[:, :, :, None].broadcast_to([P, T, 2, Q]),
                    OP.is_equal,
                )

                # ---- loc path on gpsimd (bf16), strip reduce on vector ----
                df = small.tile([128, T, 4], BF16, tag="df")
                nc.gpsimd.tensor_tensor(
                    df[:P], aux_t[:P, :, 0:4], aux_t[:P, :, 4:8], OP.subtract
                )
                ad = small.tile([128, T, 4], BF16, tag="ad")
                nc.scalar.activation(ad[:P], df[:P], AF.Abs)
                dm = small.tile([128, T, 4], BF16, tag="dm")
                nc.vector.tensor_scalar_min(dm[:P], ad[:P], 1.0)
                r_t = small.tile([128, T, 4], BF16, tag="r")
                nc.gpsimd.tensor_tensor(r_t[:P], ad[:P], dm[:P], OP.subtract)
                q_t = small.tile([128, T, 4], BF16, tag="q")
                nc.gpsimd.tensor_tensor(q_t[:P], dm[:P], dm[:P], OP.mult)
                sl1 = small.tile([128, T, 4], BF16, tag="sl1")
                nc.vector.scalar_tensor_tensor(
                    sl1[:P], q_t[:P], 0.5, r_t[:P], OP.mult, OP.add
                )
                slm = small.tile([128, T, 4], BF16, tag="slm")
                nc.gpsimd.tensor_tensor(
                    slm[:P], sl1[:P], posv.broadcast_to([P, T, 4]), OP.mult
                )
                with nc.allow_low_precision("partial sums accumulate in f32 strip"):
                    nc.vector.reduce_sum(strip[:P, i : i + 1], slm[:P], axis=AX.XY)

                if i + PREFETCH < n_tiles:
                    dispatch(i + PREFETCH)

                if i in PB_AFTER:
                    phase_b(PB_AFTER[i], i + 1)

            # ---- finalize: combined [128, 19] output, single DMA ----
            fin = singles.tile([128, 2 * Q + 1], F32)
            nc.vector.reduce_sum(fin[:, 2 * Q : 2 * Q + 1], strip[:, :], axis=AX.X)
            nc.vector.tensor_copy(fin[0:Q, 0 : 2 * Q], ph_banks[0][:, :])
            for k in range(1, NB):
                nc.vector.tensor_tensor(
                    fin[0:Q, 0 : 2 * Q], fin[0:Q, 0 : 2 * Q], ph_banks[k][:, :], OP.add
                )
            nc.gpsimd.dma_start(outt[:, :], fin[:, :])

    nc.compile()
    return nc


_CACHED = {}


def _get_nc(A, APP, T):
    key = (A, APP, T)
    if key not in _CACHED:
        _CACHED[key] = build_kernel(A, APP, T)
    return _CACHED[key]


def combine_host(hists, locsums, alpha):
    """hists: [ncores, 9, 18]; locsums: [ncores, 128, 1]; alpha: [81]."""
    hcnt = hists[:, :, 0:Q].sum(axis=0).astype(np.float64)     # [q, r]
    hw = hists[:, :, Q : 2 * Q].sum(axis=0).astype(np.float64)
    h = hw.ravel()[:C]        # h[9q+r]
    cnt = hcnt.ravel()[:C]
    alpha = alpha.astype(np.float64)
    denom = np.clip(alpha * cnt, 1.0, None)
    conf_loss = np.sum(alpha * h / denom)
    num_pos = cnt[1:].sum()
    loc_sum = locsums.astype(np.float64).sum()
    denom_loc = max(num_pos * 4.0, 1.0)
    loc_loss = loc_sum / denom_loc if num_pos > 0 else 0.0
    return np.float32(loc_loss), np.float32(conf_loss)


def kernel(loc_pred, conf_pred, targets, alpha, _trace=False):
    B, A, _ = conf_pred.shape
    assert B == 8 and A == 76725
    AP_ = 76800  # pad to 128*600 so every tile covers all 128 partitions
    nc = _get_nc(AP_, 600, 75)

    conf16 = np.zeros((B, AP_, C), dtype=BF16NP)               # [B, AP_, 81]
    conf16[:, :A] = np.asarray(conf_pred, dtype=BF16NP)
    tgt = np.asarray(targets, dtype=np.float32)
    lab_i = np.full((B, AP_), -1, dtype=np.int32)
    lab_i[:, :A] = tgt[:, :, 4].astype(np.int32)               # [B, AP_]
    labq = lab_i // 9
    labr = lab_i - 9 * labq
    xsel = np.take_along_axis(
        conf16, np.maximum(lab_i, 0)[:, :, None], axis=2
    )[:, :, 0]                                                 # [B, AP_] bf16

    aux = np.zeros((B, AP_, 12), dtype=BF16NP)
    aux[:, :A, 0:4] = loc_pred
    aux[:, :A, 4:8] = tgt[:, :, 0:4]
    aux[:, :, 8] = (lab_i > 0)
    aux[:, :, 9] = labq
    aux[:, :, 10] = labr
    aux[:, :, 11] = xsel

    in_maps = [
        {
            "conf": np.ascontiguousarray(conf16[b]),
            "aux": np.ascontiguousarray(aux[b]),
        }
        for b in range(B)
    ]
    res = run_bass_kernel_spmd(nc, in_maps, core_ids=list(range(B)), trace=_trace)
    hists = np.stack([r["outt"][0:Q, 0 : 2 * Q] for r in res.results])
    locsums = np.stack([r["outt"][:, 2 * Q] for r in res.results])
    out = combine_host(hists, locsums, np.asarray(alpha, dtype=np.float32))
    if _trace:
        return out, res
    return out


# revision 32
# speedup vs baseline: 1.5365x; 1.0301x over previous
"""Focal-loss + smooth-L1 loss kernel for TRN2, SPMD over 8 NeuronCores.

Sharding: data-parallel over the batch axis (B=8 -> one batch row per core).

Host prep (per core), all bf16:
  conf16 [A, 81]  - logits
  aux   [A, 12]   - loc(4), box(4), lab, labq=lab//9, labr=lab-9*labq,
                    xsel=conf[n, max(lab,0)]
Device (per core, anchor n = 600*p + t; tiles of T=75, last tile P=127):
  phase A per tile (pipelined):
    e[:, :, 0:81] = exp(conf)       (scalar engine; e rows padded to 96 with
                                     persistent zero pad cols for the fold)
    s = fold-tree sum_c e (96->48->24->12->6->3->reduce)  (vector, 2x bf16)
    aq -> rhs_all[.., 9:18], ar -> ar_all, xsel -> xsel_all
    smooth-L1 partials on gpsimd, strip reduce on vector
  phase B per quad of tiles (batches ACT table switches):
    lns = ln(s); pt = exp(xsel - lns); w0 = (1-pt)^2*(lns-xsel)
    rhs_all[.., 0:9] = aq * w0
    per-t matmul ph[r, k] += ar_t^T @ rhs_t -> PSUM [9, 18]
Host combine: h[9q+r] = ph[r, q], cnt[9q+r] = ph[r, 9+q]; tiny final math.

All bulk HBM->SBUF transfers go through SWDGE (gpsimd) so descriptors
spread across all 16 SDMA engines (HWDGE pins them to one engine).
"""

import numpy as np
import ml_dtypes

import concourse.bass as bass
import concourse.bacc as bacc
import concourse.mybir as mybir
import concourse.tile as tile
from concourse.bass_utils import run_bass_kernel_spmd

BF16NP = np.dtype(ml_dtypes.bfloat16)

F32 = mybir.dt.float32
BF16 = mybir.dt.bfloat16
I16 = mybir.dt.int16
AF = mybir.ActivationFunctionType
OP = mybir.AluOpType
AX = mybir.AxisListType

C = 81
CP = 96  # padded e-row width (even fold widths: 96/48/24/12/6/3)
Q = 9    # base-9 split: class c = 9*q + r
QUAD = 2  # tiles per phase-B batch


def build_kernel(A, APP, T):
    """A anchors (padded so A == 128*APP -> every tile uses all 128 partitions)."""
    n_tiles = APP // T
    assert A == 128 * APP, (A, APP)

    nc = bacc.Bacc(None, target_bir_lowering=False)
    conf = nc.dram_tensor("conf", [A, C], BF16, kind="ExternalInput")
    aux = nc.dram_tensor("aux", [A, 12], BF16, kind="ExternalInput")
    outt = nc.dram_tensor("outt", [128, 2 * Q + 1], F32, kind="ExternalOutput")

    with tile.TileContext(nc) as tc:
        with (
            tc.tile_pool(name="singles", bufs=1) as singles,
            tc.tile_pool(name="io", bufs=4) as io,
            tc.tile_pool(name="small", bufs=3) as small,
            tc.tile_pool(name="psum", bufs=1, space="PSUM") as psum,
        ):
            # constants / persistent accumulators
            iota_i = singles.tile([128, Q], I16)
            nc.gpsimd.iota(iota_i[:, :], [[1, Q]], channel_multiplier=0)
            iota9 = singles.tile([128, Q], BF16)
            nc.vector.tensor_copy(iota9[:, :], iota_i[:, :])
            ones1 = singles.tile([128, 1], BF16)
            nc.vector.memset(ones1[:, :], 1.0)
            negone = singles.tile([128, 1], F32)
            nc.vector.memset(negone[:, :], -1.0)

            NEB = 3
            e_bufs = [
                singles.tile([128, T, CP], BF16, tag=f"e{k}", name=f"e{k}")
                for k in range(NEB)
            ]
            for k in range(NEB):
                nc.vector.memset(e_bufs[k][:, :, C:CP], 0.0)

            s_all = singles.tile([128, APP], F32)
            nc.vector.memset(s_all[:, :], 1.0)
            u_all = singles.tile([128, APP], F32)
            lns_all = singles.tile([128, APP], F32)
            pt_all = singles.tile([128, APP], F32)
            usq_all = singles.tile([128, APP], F32)
            w0_all = singles.tile([128, APP], BF16)
            # comb planes: [.., 0, :] = aq, [.., 1, :] = ar, [.., 2, :] = ar*w0
            comb_all = singles.tile([128, APP, 3, Q], BF16)
            iota18 = singles.tile([128, 2 * Q], BF16)
            nc.vector.tensor_copy(iota18[:, 0:Q], iota_i[:, :])
            nc.vector.tensor_copy(iota18[:, Q : 2 * Q], iota_i[:, :])

            ph_loc = psum.tile([1, 300], F32, name="ph_loc")
            NB = 7  # PSUM banks (1 reserved for ph_loc) round-robin
            ph_banks = [
                psum.tile([Q, 2 * Q], F32, tag=f"ph{k}", name=f"ph{k}")
                for k in range(NB)
            ]

            def phase_b(i0, i1):
                """Ln/exp batch + matmuls for tiles i0..i1-1."""
                g0 = i0 * T
                g1 = i1 * T
                W = g1 - g0
                sl = slice(g0, g1)
                nc.scalar.activation(lns_all[:, sl], s_all[:, sl], AF.Ln)
                for j in range(i0, i1):
                    jaux = live_aux[j]
                    nc.vector.tensor_tensor(
                        u_all[:, j * T : (j + 1) * T],
                        jaux[:, :, 11:12].squeeze(),
                        lns_all[:, j * T : (j + 1) * T],
                        OP.subtract,
                    )
                nc.scalar.activation(pt_all[:, sl], u_all[:, sl], AF.Exp)
                nc.scalar.activation(
                    usq_all[:, sl], pt_all[:, sl], AF.Square, bias=negone[:, :]
                )  # (pt - 1)^2
                nc.vector.scalar_tensor_tensor(
                    w0_all[:, sl], u_all[:, sl], -1.0, usq_all[:, sl],
                    OP.mult, OP.mult,
                )  # w0 = (lns - xsel) * (1-pt)^2
                nc.vector.tensor_tensor(
                    comb_all[:, sl, 2, :],
                    comb_all[:, sl, 1, :],
                    w0_all[:, sl, None].broadcast_to([128, W, Q]),
                    OP.mult,
                )
                for tg in range(g0, g1):
                    nc.tensor.matmul(
                        ph_banks[tg % NB][:, :],
                        comb_all[:, tg, 0, :],
                        comb_all[:, tg, 1:3, :],
                        start=(tg < NB),
                        stop=(tg >= APP - NB),
                    )

            # phase-B groups: pairs early (amortize ACT table loads),
            # singles at the end (short critical-path tail)
            PB_AFTER = {1: 0, 3: 2, 5: 4, 6: 6, 7: 7}
            PREFETCH = 3
            pending = {}
            live_aux = {}

            def dispatch(j):
                jt0 = j * T
                conf_t = io.tile([128, T, C], BF16, tag="conf", name="conf_t")
                nc.gpsimd.dma_start(
                    conf_t[:, :, :],
                    bass.AP(
                        tensor=conf[:, :].tensor,
                        offset=jt0 * C,
                        ap=[[APP * C, 128], [C, T], [1, C]],
                    ),
                )
                aux_t = io.tile([128, T, 12], BF16, tag="aux", name="aux_t")
                nc.gpsimd.dma_start(
                    aux_t[:, :, :],
                    bass.AP(
                        tensor=aux[:, :].tensor,
                        offset=jt0 * 12,
                        ap=[[APP * 12, 128], [12, T], [1, 12]],
                    ),
                )
                pending[j] = (conf_t, aux_t)

            for j in range(PREFETCH):
                dispatch(j)

            for i in range(n_tiles):
                t0 = i * T
                P = 128
                ts = slice(t0, t0 + T)

                conf_t, aux_t = pending.pop(i)

                posv = aux_t[:P, :, 8:9]
                labqr = aux_t[:P, :, 9:11]

                # ---- conf path ----
                e_t = e_bufs[i % NEB]
                nc.scalar.activation(e_t[:P, :, 0:C], conf_t[:P], AF.Exp)
                w = CP
                while w > 3:
                    h = w // 2
                    nc.vector.tensor_tensor(
                        e_t[:P, :, 0:h], e_t[:P, :, 0:h], e_t[:P, :, h:w], OP.add
                    )
                    w = h
                nc.vector.reduce_sum(s_all[:P, ts], e_t[:P, :, 0:3], axis=AX.X)

                live_aux[i] = aux_t

                nc.vector.tensor_tensor(
                    comb_all[:P, ts, 0:2, :],
                    iota18[:P, None, :].broadcast_to([P, T, 2 * Q]),
                    labqr[:, :, :, None].broadcast_to([P, T, 2, Q]),
                    OP.is_equal,
                )

                # ---- loc path on gpsimd (bf16), strip reduce on vector ----
                df = small.tile([128, T, 4], BF16, tag="df")
                nc.gpsimd.tensor_tensor(
                    df[:P], aux_t[:P, :, 0:4], aux_t[:P, :, 4:8], OP.subtract
                )
                ad = small.tile([128, T, 4], BF16, tag="ad")
                nc.scalar.activation(ad[:P], df[:P], AF.Abs)
                dm = small.tile([128, T, 4], BF16, tag="dm")
                nc.vector.tensor_scalar_min(dm[:P], ad[:P], 1.0)
                r_t = small.tile([128, T, 4], BF16, tag="r")
                nc.gpsimd.tensor_tensor(r_t[:P], ad[:P], dm[:P], OP.subtract)
                q_t = small.tile([128, T, 4], BF16, tag="q")
                nc.gpsimd.tensor_tensor(q_t[:P], dm[:P], dm[:P], OP.mult)
                sl1 = small.tile([128, T, 4], BF16, tag="sl1")
                nc.vector.scalar_tensor_tensor(
                    sl1[:P], q_t[:P], 0.5, r_t[:P], OP.mult, OP.add
                )
                slm = small.tile([128, T, 4], BF16, tag="slm")
                nc.gpsimd.tensor_tensor(
                    slm[:P], sl1[:P], posv.broadcast_to([P, T, 4]), OP.mult
                )
                nc.tensor.matmul(
                    ph_loc[:, :],
                    ones1[:P, :],
                    slm[:P],
                    start=(i == 0),
                    stop=(i == n_tiles - 1),
                )

                if i in PB_AFTER:
                    phase_b(PB_AFTER[i], i + 1)

                if i + PREFETCH < n_tiles:
                    dispatch(i + PREFETCH)

            # ---- finalize: combined [128, 19] output, single DMA ----
            fin = singles.tile([128, 2 * Q + 1], F32)
            nc.vector.reduce_sum(fin[0:1, 2 * Q : 2 * Q + 1], ph_loc[:, :], axis=AX.X)
            nc.vector.tensor_copy(fin[0:Q, 0 : 2 * Q], ph_banks[0][:, :])
            for k in range(1, NB):
                nc.vector.tensor_tensor(
                    fin[0:Q, 0 : 2 * Q], fin[0:Q, 0 : 2 * Q], ph_banks[k][:, :], OP.add
                )
            nc.gpsimd.dma_start(outt[:, :], fin[:, :])

    nc.compile()
    return nc


_CACHED = {}


def _get_nc(A, APP, T):
    key = (A, APP, T)
    if key not in _CACHED:
        _CACHED[key] = build_kernel(A, APP, T)
    return _CACHED[key]


def combine_host(hists, locsums, alpha):
    """hists: [ncores, 9, 18]; locsums: [ncores, 128, 1]; alpha: [81]."""
    hcnt = hists[:, :, 0:Q].sum(axis=0).astype(np.float64)     # [q, r]
    hw = hists[:, :, Q : 2 * Q].sum(axis=0).astype(np.float64)
    h = hw.ravel()[:C]        # h[9q+r]
    cnt = hcnt.ravel()[:C]
    alpha = alpha.astype(np.float64)
    denom = np.clip(alpha * cnt, 1.0, None)
    conf_loss = np.sum(alpha * h / denom)
    num_pos = cnt[1:].sum()
    loc_sum = locsums.astype(np.float64).sum()
    denom_loc = max(num_pos * 4.0, 1.0)
    loc_loss = loc_sum / denom_loc if num_pos > 0 else 0.0
    return np.float32(loc_loss), np.float32(conf_loss)


def kernel(loc_pred, conf_pred, targets, alpha, _trace=False):
    B, A, _ = conf_pred.shape
    assert B == 8 and A == 76725
    AP_ = 76800  # pad to 128*600 so every tile covers all 128 partitions
    nc = _get_nc(AP_, 600, 75)

    conf16 = np.zeros((B, AP_, C), dtype=BF16NP)               # [B, AP_, 81]
    conf16[:, :A] = np.asarray(conf_pred, dtype=BF16NP)
    tgt = np.asarray(targets, dtype=np.float32)
    lab_i = np.full((B, AP_), -1, dtype=np.int32)
    lab_i[:, :A] = tgt[:, :, 4].astype(np.int32)               # [B, AP_]
    labq = lab_i // 9
    labr = lab_i - 9 * labq
    xsel = np.take_along_axis(
        conf16, np.maximum(lab_i, 0)[:, :, None], axis=2
    )[:, :, 0]                                                 # [B, AP_] bf16

    aux = np.zeros((B, AP_, 12), dtype=BF16NP)
    aux[:, :A, 0:4] = loc_pred
    aux[:, :A, 4:8] = tgt[:, :, 0:4]
    aux[:, :, 8] = (lab_i > 0)
    aux[:, :, 9] = labq
    aux[:, :, 10] = labr
    aux[:, :, 11] = xsel

    in_maps = [
        {
            "conf": np.ascontiguousarray(conf16[b]),
            "aux": np.ascontiguousarray(aux[b]),
        }
        for b in range(B)
    ]
    res = run_bass_kernel_spmd(nc, in_maps, core_ids=list(range(B)), trace=_trace)
    hists = np.stack([r["outt"][0:Q, 0 : 2 * Q] for r in res.results])
    locsums = np.stack([r["outt"][0, 2 * Q] for r in res.results])
    out = combine_host(hists, locsums, np.asarray(alpha, dtype=np.float32))
    if _trace:
        return out, res
    return out


# revision 33
# speedup vs baseline: 1.6301x; 1.0609x over previous
"""Focal-loss + smooth-L1 loss kernel for TRN2, SPMD over 8 NeuronCores.

Sharding: data-parallel over the batch axis (B=8 -> one batch row per core).

Host prep (per core), all bf16:
  conf16 [A, 81]  - logits
  aux   [A, 12]   - loc(4), box(4), lab, labq=lab//9, labr=lab-9*labq,
                    xsel=conf[n, max(lab,0)]
Device (per core, anchor n = 600*p + t; tiles of T=75, last tile P=127):
  phase A per tile (pipelined):
    e[:, :, 0:81] = exp(conf)       (scalar engine; e rows padded to 96 with
                                     persistent zero pad cols for the fold)
    s = fold-tree sum_c e (96->48->24->12->6->3->reduce)  (vector, 2x bf16)
    aq -> rhs_all[.., 9:18], ar -> ar_all, xsel -> xsel_all
    smooth-L1 partials on gpsimd, strip reduce on vector
  phase B per quad of tiles (batches ACT table switches):
    lns = ln(s); pt = exp(xsel - lns); w0 = (1-pt)^2*(lns-xsel)
    rhs_all[.., 0:9] = aq * w0
    per-t matmul ph[r, k] += ar_t^T @ rhs_t -> PSUM [9, 18]
Host combine: h[9q+r] = ph[r, q], cnt[9q+r] = ph[r, 9+q]; tiny final math.

All bulk HBM->SBUF transfers go through SWDGE (gpsimd) so descriptors
spread across all 16 SDMA engines (HWDGE pins them to one engine).
"""

import numpy as np
import ml_dtypes

import concourse.bass as bass
import concourse.bacc as bacc
import concourse.mybir as mybir
import concourse.tile as tile
from concourse.bass_utils import run_bass_kernel_spmd

BF16NP = np.dtype(ml_dtypes.bfloat16)

F32 = mybir.dt.float32
BF16 = mybir.dt.bfloat16
I16 = mybir.dt.int16
AF = mybir.ActivationFunctionType
OP = mybir.AluOpType
AX = mybir.AxisListType

C = 81
CP = 96  # padded e-row width (even fold widths: 96/48/24/12/6/3)
Q = 9    # base-9 split: class c = 9*q + r
QUAD = 2  # tiles per phase-B batch


def build_kernel(A, APP, T):
    """A anchors (padded so A == 128*APP -> every tile uses all 128 partitions)."""
    n_tiles = APP // T
    assert A == 128 * APP, (A, APP)

    nc = bacc.Bacc(None, target_bir_lowering=False)
    conf = nc.dram_tensor("conf", [A, C], BF16, kind="ExternalInput")
    aux = nc.dram_tensor("aux", [A, 10], BF16, kind="ExternalInput")
    qr = nc.dram_tensor("qr", [A, 2 * Q], BF16, kind="ExternalInput")
    outt = nc.dram_tensor("outt", [128, Q + 1], F32, kind="ExternalOutput")

    with tile.TileContext(nc) as tc:
        with (
            tc.tile_pool(name="singles", bufs=1) as singles,
            tc.tile_pool(name="io", bufs=4) as io,
            tc.tile_pool(name="small", bufs=3) as small,
            tc.tile_pool(name="psum", bufs=1, space="PSUM") as psum,
        ):
            # constants / persistent accumulators
            iota_i = singles.tile([128, Q], I16)
            nc.gpsimd.iota(iota_i[:, :], [[1, Q]], channel_multiplier=0)
            iota9 = singles.tile([128, Q], BF16)
            nc.vector.tensor_copy(iota9[:, :], iota_i[:, :])
            ones1 = singles.tile([128, 1], BF16)
            nc.vector.memset(ones1[:, :], 1.0)
            negone = singles.tile([128, 1], F32)
            nc.vector.memset(negone[:, :], -1.0)

            NEB = 3
            e_bufs = [
                singles.tile([128, T, CP], BF16, tag=f"e{k}", name=f"e{k}")
                for k in range(NEB)
            ]
            for k in range(NEB):
                nc.vector.memset(e_bufs[k][:, :, C:CP], 0.0)

            s_all = singles.tile([128, APP], F32)
            nc.vector.memset(s_all[:, :], 1.0)
            u_all = singles.tile([128, APP], F32)
            lns_all = singles.tile([128, APP], F32)
            pt_all = singles.tile([128, APP], F32)
            usq_all = singles.tile([128, APP], F32)
            w0_all = singles.tile([128, APP], BF16)

            ph_loc = psum.tile([1, 300], F32, name="ph_loc")
            NB = 7  # PSUM banks (1 reserved for ph_loc) round-robin
            ph_banks = [
                psum.tile([Q, Q], F32, tag=f"ph{k}", name=f"ph{k}")
                for k in range(NB)
            ]

            def phase_b(i0, i1):
                """Ln/exp batch + matmuls for tiles i0..i1-1."""
                g0 = i0 * T
                g1 = i1 * T
                W = g1 - g0
                sl = slice(g0, g1)
                nc.scalar.activation(lns_all[:, sl], s_all[:, sl], AF.Ln)
                for j in range(i0, i1):
                    jaux = live_aux[j]
                    nc.vector.tensor_tensor(
                        u_all[:, j * T : (j + 1) * T],
                        jaux[:, :, 9:10].squeeze(),
                        lns_all[:, j * T : (j + 1) * T],
                        OP.subtract,
                    )
                nc.scalar.activation(pt_all[:, sl], u_all[:, sl], AF.Exp)
                nc.scalar.activation(
                    usq_all[:, sl], pt_all[:, sl], AF.Square, bias=negone[:, :]
                )  # (pt - 1)^2
                nc.vector.scalar_tensor_tensor(
                    w0_all[:, sl], u_all[:, sl], -1.0, usq_all[:, sl],
                    OP.mult, OP.mult,
                )  # w0 = (lns - xsel) * (1-pt)^2
                for j in range(i0, i1):
                    jqr = live_qr[j]
                    nc.vector.tensor_tensor(
                        jqr[:, :, Q : 2 * Q],
                        jqr[:, :, Q : 2 * Q],
                        w0_all[:, j * T : (j + 1) * T, None].broadcast_to(
                            [128, T, Q]
                        ),
                        OP.mult,
                    )
                for tg in range(g0, g1):
                    jqr = live_qr[tg // T]
                    tl = tg % T
                    nc.tensor.matmul(
                        ph_banks[tg % NB][:, :],
                        jqr[:, tl, 0:Q],
                        jqr[:, tl, Q : 2 * Q],
                        start=(tg < NB),
                        stop=(tg >= APP - NB),
                    )

            # phase-B groups: pairs early (amortize ACT table loads),
            # singles at the end (short critical-path tail)
            PB_AFTER = {1: 0, 3: 2, 5: 4, 6: 6, 7: 7}
            PREFETCH = 3
            pending = {}
            live_aux = {}
            live_qr = {}

            def dispatch(j):
                jt0 = j * T
                conf_t = io.tile([128, T, C], BF16, tag="conf", name="conf_t")
                nc.gpsimd.dma_start(
                    conf_t[:, :, :],
                    bass.AP(
                        tensor=conf[:, :].tensor,
                        offset=jt0 * C,
                        ap=[[APP * C, 128], [C, T], [1, C]],
                    ),
                )
                aux_t = io.tile([128, T, 10], BF16, tag="aux", name="aux_t")
                nc.gpsimd.dma_start(
                    aux_t[:, :, :],
                    bass.AP(
                        tensor=aux[:, :].tensor,
                        offset=jt0 * 10,
                        ap=[[APP * 10, 128], [10, T], [1, 10]],
                    ),
                )
                qr_t = io.tile([128, T, 2 * Q], BF16, tag="qr", name="qr_t")
                nc.gpsimd.dma_start(
                    qr_t[:, :, :],
                    bass.AP(
                        tensor=qr[:, :].tensor,
                        offset=jt0 * 2 * Q,
                        ap=[[APP * 2 * Q, 128], [2 * Q, T], [1, 2 * Q]],
                    ),
                )
                pending[j] = (conf_t, aux_t, qr_t)

            for j in range(PREFETCH):
                dispatch(j)

            for i in range(n_tiles):
                t0 = i * T
                P = 128
                ts = slice(t0, t0 + T)

                conf_t, aux_t, qr_t = pending.pop(i)

                posv = aux_t[:P, :, 8:9]

                # ---- conf path ----
                e_t = e_bufs[i % NEB]
                nc.scalar.activation(e_t[:P, :, 0:C], conf_t[:P], AF.Exp)
                w = CP
                while w > 3:
                    h = w // 2
                    nc.vector.tensor_tensor(
                        e_t[:P, :, 0:h], e_t[:P, :, 0:h], e_t[:P, :, h:w], OP.add
                    )
                    w = h
                nc.vector.reduce_sum(s_all[:P, ts], e_t[:P, :, 0:3], axis=AX.X)

                live_aux[i] = aux_t
                live_qr[i] = qr_t

                # ---- loc path on gpsimd (bf16), strip reduce on vector ----
                df = small.tile([128, T, 4], BF16, tag="df")
                nc.gpsimd.tensor_tensor(
                    df[:P], aux_t[:P, :, 0:4], aux_t[:P, :, 4:8], OP.subtract
                )
                ad = small.tile([128, T, 4], BF16, tag="ad")
                nc.scalar.activation(ad[:P], df[:P], AF.Abs)
                dm = small.tile([128, T, 4], BF16, tag="dm")
                nc.vector.tensor_scalar_min(dm[:P], ad[:P], 1.0)
                r_t = small.tile([128, T, 4], BF16, tag="r")
                nc.gpsimd.tensor_tensor(r_t[:P], ad[:P], dm[:P], OP.subtract)
                q_t = small.tile([128, T, 4], BF16, tag="q")
                nc.gpsimd.tensor_tensor(q_t[:P], dm[:P], dm[:P], OP.mult)
                sl1 = small.tile([128, T, 4], BF16, tag="sl1")
                nc.vector.scalar_tensor_tensor(
                    sl1[:P], q_t[:P], 0.5, r_t[:P], OP.mult, OP.add
                )
                slm = small.tile([128, T, 4], BF16, tag="slm")
                nc.gpsimd.tensor_tensor(
                    slm[:P], sl1[:P], posv.broadcast_to([P, T, 4]), OP.mult
                )
                nc.tensor.matmul(
                    ph_loc[:, :],
                    ones1[:P, :],
                    slm[:P],
                    start=(i == 0),
                    stop=(i == n_tiles - 1),
                )

                if i in PB_AFTER:
                    phase_b(PB_AFTER[i], i + 1)

                if i + PREFETCH < n_tiles:
                    dispatch(i + PREFETCH)

            # ---- finalize: combined [128, 10] output, single DMA ----
            fin = singles.tile([128, Q + 1], F32)
            nc.vector.reduce_sum(fin[0:1, Q : Q + 1], ph_loc[:, :], axis=AX.X)
            nc.vector.tensor_copy(fin[0:Q, 0:Q], ph_banks[0][:, :])
            for k in range(1, NB):
                nc.vector.tensor_tensor(
                    fin[0:Q, 0:Q], fin[0:Q, 0:Q], ph_banks[k][:, :], OP.add
                )
            nc.gpsimd.dma_start(outt[:, :], fin[:, :])

    nc.compile()
    return nc


_CACHED = {}


def _get_nc(A, APP, T):
    key = (A, APP, T)
    if key not in _CACHED:
        _CACHED[key] = build_kernel(A, APP, T)
    return _CACHED[key]


def combine_host(hists, locsums, cnt, alpha):
    """hists: [ncores, 9, 9]; locsums: [ncores]; cnt: [81]; alpha: [81]."""
    h = hists.sum(axis=0).astype(np.float64).ravel()[:C]   # h[9q+r]
    cnt = cnt.astype(np.float64)
    alpha = alpha.astype(np.float64)
    denom = np.clip(alpha * cnt, 1.0, None)
    conf_loss = np.sum(alpha * h / denom)
    num_pos = cnt[1:].sum()
    loc_sum = locsums.astype(np.float64).sum()
    denom_loc = max(num_pos * 4.0, 1.0)
    loc_loss = loc_sum / denom_loc if num_pos > 0 else 0.0
    return np.float32(loc_loss), np.float32(conf_loss)


def kernel(loc_pred, conf_pred, targets, alpha, _trace=False):
    B, A, _ = conf_pred.shape
    assert B == 8 and A == 76725
    AP_ = 76800  # pad to 128*600 so every tile covers all 128 partitions
    nc = _get_nc(AP_, 600, 75)

    conf16 = np.zeros((B, AP_, C), dtype=BF16NP)               # [B, AP_, 81]
    conf16[:, :A] = np.asarray(conf_pred, dtype=BF16NP)
    tgt = np.asarray(targets, dtype=np.float32)
    lab_i = np.full((B, AP_), -1, dtype=np.int32)
    lab_i[:, :A] = tgt[:, :, 4].astype(np.int32)               # [B, AP_]
    labq = lab_i // 9
    labr = lab_i - 9 * labq
    xsel = np.take_along_axis(
        conf16, np.maximum(lab_i, 0)[:, :, None], axis=2
    )[:, :, 0]                                                 # [B, AP_] bf16
    cnt = np.bincount(lab_i[lab_i >= 0], minlength=C)[:C]

    aux = np.zeros((B, AP_, 10), dtype=BF16NP)
    aux[:, :A, 0:4] = loc_pred
    aux[:, :A, 4:8] = tgt[:, :, 0:4]
    aux[:, :, 8] = (lab_i > 0)
    aux[:, :, 9] = xsel

    # one-hot planes: qr[.., 0:9] = 1[labq=q], qr[.., 9:18] = 1[labr=r]
    qr = np.zeros((B, AP_, 2 * Q), dtype=BF16NP)
    iq = np.arange(Q)
    qr[:, :, 0:Q] = (labq[:, :, None] == iq)
    qr[:, :, Q : 2 * Q] = (labr[:, :, None] == iq) * (labq[:, :, None] >= 0)

    in_maps = [
        {
            "conf": np.ascontiguousarray(conf16[b]),
            "aux": np.ascontiguousarray(aux[b]),
            "qr": np.ascontiguousarray(qr[b]),
        }
        for b in range(B)
    ]
    res = run_bass_kernel_spmd(nc, in_maps, core_ids=list(range(B)), trace=_trace)
    hists = np.stack([r["outt"][0:Q, 0:Q] for r in res.results])
    locsums = np.stack([r["outt"][0, Q] for r in res.results])
    out = combine_host(hists, locsums, cnt, np.asarray(alpha, dtype=np.float32))
    if _trace:
        return out, res
    return out


# revision 34
# speedup vs baseline: 1.6689x; 1.0238x over previous
"""Focal-loss + smooth-L1 loss kernel for TRN2, SPMD over 8 NeuronCores.

Sharding: data-parallel over the batch axis (B=8 -> one batch row per core).

Host prep (per core), all bf16:
  conf16 [A, 81]  - logits
  aux   [A, 12]   - loc(4), box(4), lab, labq=lab//9, labr=lab-9*labq,
                    xsel=conf[n, max(lab,0)]
Device (per core, anchor n = 600*p + t; tiles of T=75, last tile P=127):
  phase A per tile (pipelined):
    e[:, :, 0:81] = exp(conf)       (scalar engine; e rows padded to 96 with
                                     persistent zero pad cols for the fold)
    s = fold-tree sum_c e (96->48->24->12->6->3->reduce)  (vector, 2x bf16)
    aq -> rhs_all[.., 9:18], ar -> ar_all, xsel -> xsel_all
    smooth-L1 partials on gpsimd, strip reduce on vector
  phase B per quad of tiles (batches ACT table switches):
    lns = ln(s); pt = exp(xsel - lns); w0 = (1-pt)^2*(lns-xsel)
    rhs_all[.., 0:9] = aq * w0
    per-t matmul ph[r, k] += ar_t^T @ rhs_t -> PSUM [9, 18]
Host combine: h[9q+r] = ph[r, q], cnt[9q+r] = ph[r, 9+q]; tiny final math.

All bulk HBM->SBUF transfers go through SWDGE (gpsimd) so descriptors
spread across all 16 SDMA engines (HWDGE pins them to one engine).
"""

import numpy as np
import ml_dtypes

import concourse.bass as bass
import concourse.bacc as bacc
import concourse.mybir as mybir
import concourse.tile as tile
from concourse.bass_utils import run_bass_kernel_spmd

BF16NP = np.dtype(ml_dtypes.bfloat16)

F32 = mybir.dt.float32
BF16 = mybir.dt.bfloat16
I16 = mybir.dt.int16
AF = mybir.ActivationFunctionType
OP = mybir.AluOpType
AX = mybir.AxisListType

C = 81
CP = 96  # padded e-row width (even fold widths: 96/48/24/12/6/3)
Q = 9    # base-9 split: class c = 9*q + r
QUAD = 2  # tiles per phase-B batch


def build_kernel(A, APP, T):
    """A anchors (padded so A == 128*APP -> every tile uses all 128 partitions)."""
    n_tiles = APP // T
    assert A == 128 * APP, (A, APP)

    nc = bacc.Bacc(None, target_bir_lowering=False)
    conf = nc.dram_tensor("conf", [A, C], BF16, kind="ExternalInput")
    aux = nc.dram_tensor("aux", [A, 10], BF16, kind="ExternalInput")
    qr = nc.dram_tensor("qr", [A, 2 * Q], BF16, kind="ExternalInput")
    outt = nc.dram_tensor("outt", [128, Q + 1], F32, kind="ExternalOutput")

    with tile.TileContext(nc) as tc:
        with (
            tc.tile_pool(name="singles", bufs=1) as singles,
            tc.tile_pool(name="io", bufs=4) as io,
            tc.tile_pool(name="small", bufs=3) as small,
            tc.tile_pool(name="psum", bufs=1, space="PSUM") as psum,
        ):
            # constants / persistent accumulators
            ones1 = singles.tile([128, 1], BF16)
            negone = singles.tile([128, 1], F32)

            NEB = 3
            e_bufs = [
                singles.tile([128, T, CP], BF16, tag=f"e{k}", name=f"e{k}")
                for k in range(NEB)
            ]
            for k in range(NEB):
                nc.vector.memset(e_bufs[k][:, :, C:CP], 0.0)

            s_all = singles.tile([128, APP], F32)
            nc.vector.memset(s_all[:, :], 1.0)
            u_all = singles.tile([128, APP], F32)
            lns_all = singles.tile([128, APP], F32)
            pt_all = singles.tile([128, APP], F32)
            usq_all = singles.tile([128, APP], F32)
            w0_all = singles.tile([128, APP], BF16)

            ph_loc = psum.tile([1, 300], F32, name="ph_loc")
            NB = 7  # PSUM banks (1 reserved for ph_loc) round-robin
            ph_banks = [
                psum.tile([Q, Q], F32, tag=f"ph{k}", name=f"ph{k}")
                for k in range(NB)
            ]

            def phase_b(i0, i1):
                """Ln/exp batch + matmuls for tiles i0..i1-1."""
                g0 = i0 * T
                g1 = i1 * T
                W = g1 - g0
                sl = slice(g0, g1)
                nc.scalar.activation(lns_all[:, sl], s_all[:, sl], AF.Ln)
                for j in range(i0, i1):
                    jaux = live_aux[j]
                    nc.vector.tensor_tensor(
                        u_all[:, j * T : (j + 1) * T],
                        jaux[:, :, 9:10].squeeze(),
                        lns_all[:, j * T : (j + 1) * T],
                        OP.subtract,
                    )
                nc.scalar.activation(pt_all[:, sl], u_all[:, sl], AF.Exp)
                nc.scalar.activation(
                    usq_all[:, sl], pt_all[:, sl], AF.Square, bias=negone[:, :]
                )  # (pt - 1)^2
                nc.vector.scalar_tensor_tensor(
                    w0_all[:, sl], u_all[:, sl], -1.0, usq_all[:, sl],
                    OP.mult, OP.mult,
                )  # w0 = (lns - xsel) * (1-pt)^2
                for j in range(i0, i1):
                    jqr = live_qr[j]
                    nc.vector.tensor_tensor(
                        jqr[:, :, Q : 2 * Q],
                        jqr[:, :, Q : 2 * Q],
                        w0_all[:, j * T : (j + 1) * T, None].broadcast_to(
                            [128, T, Q]
                        ),
                        OP.mult,
                    )
                for tg in range(g0, g1):
                    jqr = live_qr[tg // T]
                    tl = tg % T
                    nc.tensor.matmul(
                        ph_banks[tg % NB][:, :],
                        jqr[:, tl, 0:Q],
                        jqr[:, tl, Q : 2 * Q],
                        start=(tg < NB),
                        stop=(tg >= APP - NB),
                    )

            # phase-B groups: pairs early (amortize ACT table loads),
            # singles at the end (short critical-path tail)
            PB_AFTER = {3: 0, 5: 4, 6: 6, 7: 7}
            PREFETCH = 3
            pending = {}
            live_aux = {}
            live_qr = {}

            def dispatch(j):
                jt0 = j * T
                conf_t = io.tile([128, T, C], BF16, tag="conf", name="conf_t")
                nc.gpsimd.dma_start(
                    conf_t[:, :, :],
                    bass.AP(
                        tensor=conf[:, :].tensor,
                        offset=jt0 * C,
                        ap=[[APP * C, 128], [C, T], [1, C]],
                    ),
                )
                aux_t = io.tile([128, T, 10], BF16, tag="aux", name="aux_t")
                nc.gpsimd.dma_start(
                    aux_t[:, :, :],
                    bass.AP(
                        tensor=aux[:, :].tensor,
                        offset=jt0 * 10,
                        ap=[[APP * 10, 128], [10, T], [1, 10]],
                    ),
                )
                qr_t = io.tile([128, T, 2 * Q], BF16, tag="qr", name="qr_t")
                nc.gpsimd.dma_start(
                    qr_t[:, :, :],
                    bass.AP(
                        tensor=qr[:, :].tensor,
                        offset=jt0 * 2 * Q,
                        ap=[[APP * 2 * Q, 128], [2 * Q, T], [1, 2 * Q]],
                    ),
                )
                pending[j] = (conf_t, aux_t, qr_t)

            # tile 0: split conf into halves so the first exp starts sooner
            conf_t0 = io.tile([128, T, C], BF16, tag="conf", name="conf_t0")
            TH = T // 2
            for hh in range(2):
                r0 = hh * TH
                r1 = T if hh else TH
                nc.gpsimd.dma_start(
                    conf_t0[:, r0:r1, :],
                    bass.AP(
                        tensor=conf[:, :].tensor,
                        offset=r0 * C,
                        ap=[[APP * C, 128], [C, r1 - r0], [1, C]],
                    ),
                )
            aux_t0 = io.tile([128, T, 10], BF16, tag="aux", name="aux_t0")
            nc.gpsimd.dma_start(
                aux_t0[:, :, :],
                bass.AP(
                    tensor=aux[:, :].tensor,
                    offset=0,
                    ap=[[APP * 10, 128], [10, T], [1, 10]],
                ),
            )
            qr_t0 = io.tile([128, T, 2 * Q], BF16, tag="qr", name="qr_t0")
            nc.gpsimd.dma_start(
                qr_t0[:, :, :],
                bass.AP(
                    tensor=qr[:, :].tensor,
                    offset=0,
                    ap=[[APP * 2 * Q, 128], [2 * Q, T], [1, 2 * Q]],
                ),
            )
            pending[0] = (conf_t0, aux_t0, qr_t0)
            for j in range(1, PREFETCH):
                dispatch(j)
            # constants (issued after the prime DMAs so transfers start first)
            nc.vector.memset(ones1[:, :], 1.0)
            nc.vector.memset(negone[:, :], -1.0)

            for i in range(n_tiles):
                t0 = i * T
                P = 128
                ts = slice(t0, t0 + T)

                conf_t, aux_t, qr_t = pending.pop(i)

                posv = aux_t[:P, :, 8:9]

                # ---- conf path ----
                e_t = e_bufs[i % NEB]
                if i == 0:
                    nc.scalar.activation(
                        e_t[:P, 0:TH, 0:C], conf_t[:P, 0:TH, :], AF.Exp
                    )
                    nc.scalar.activation(
                        e_t[:P, TH:T, 0:C], conf_t[:P, TH:T, :], AF.Exp
                    )
                else:
                    nc.scalar.activation(e_t[:P, :, 0:C], conf_t[:P], AF.Exp)
                w = CP
                while w > 3:
                    h = w // 2
                    nc.vector.tensor_tensor(
                        e_t[:P, :, 0:h], e_t[:P, :, 0:h], e_t[:P, :, h:w], OP.add
                    )
                    w = h
                nc.vector.reduce_sum(s_all[:P, ts], e_t[:P, :, 0:3], axis=AX.X)

                live_aux[i] = aux_t
                live_qr[i] = qr_t

                # ---- loc path on gpsimd (bf16), strip reduce on vector ----
                df = small.tile([128, T, 4], BF16, tag="df")
                nc.gpsimd.tensor_tensor(
                    df[:P], aux_t[:P, :, 0:4], aux_t[:P, :, 4:8], OP.subtract
                )
                ad = small.tile([128, T, 4], BF16, tag="ad")
                nc.scalar.activation(ad[:P], df[:P], AF.Abs)
                dm = small.tile([128, T, 4], BF16, tag="dm")
                nc.vector.tensor_scalar_min(dm[:P], ad[:P], 1.0)
                r_t = small.tile([128, T, 4], BF16, tag="r")
                nc.gpsimd.tensor_tensor(r_t[:P], ad[:P], dm[:P], OP.subtract)
                q_t = small.tile([128, T, 4], BF16, tag="q")
                nc.gpsimd.tensor_tensor(q_t[:P], dm[:P], dm[:P], OP.mult)
                sl1 = small.tile([128, T, 4], BF16, tag="sl1")
                nc.vector.scalar_tensor_tensor(
                    sl1[:P], q_t[:P], 0.5, r_t[:P], OP.mult, OP.add
                )
                slm = small.tile([128, T, 4], BF16, tag="slm")
                nc.gpsimd.tensor_tensor(
                    slm[:P], sl1[:P], posv.broadcast_to([P, T, 4]), OP.mult
                )
                nc.tensor.matmul(
                    ph_loc[:, :],
                    ones1[:P, :],
                    slm[:P],
                    start=(i == 0),
                    stop=(i == n_tiles - 1),
                )

                if i in PB_AFTER:
                    phase_b(PB_AFTER[i], i + 1)

                if i + PREFETCH < n_tiles:
                    dispatch(i + PREFETCH)

            # ---- finalize: combined [128, 10] output, single DMA ----
            fin = singles.tile([128, Q + 1], F32)
            nc.vector.reduce_sum(fin[0:1, Q : Q + 1], ph_loc[:, :], axis=AX.X)
            nc.vector.tensor_copy(fin[0:Q, 0:Q], ph_banks[0][:, :])
            for k in range(1, NB):
                nc.vector.tensor_tensor(
                    fin[0:Q, 0:Q], fin[0:Q, 0:Q], ph_banks[k][:, :], OP.add
                )
            nc.sync.dma_start(outt[:, :], fin[:, :])

    nc.compile()
    return nc


_CACHED = {}


def _get_nc(A, APP, T):
    key = (A, APP, T)
    if key not in _CACHED:
        _CACHED[key] = build_kernel(A, APP, T)
    return _CACHED[key]


def combine_host(hists, locsums, cnt, alpha):
    """hists: [ncores, 9, 9]; locsums: [ncores]; cnt: [81]; alpha: [81]."""
    h = hists.sum(axis=0).astype(np.float64).ravel()[:C]   # h[9q+r]
    cnt = cnt.astype(np.float64)
    alpha = alpha.astype(np.float64)
    denom = np.clip(alpha * cnt, 1.0, None)
    conf_loss = np.sum(alpha * h / denom)
    num_pos = cnt[1:].sum()
    loc_sum = locsums.astype(np.float64).sum()
    denom_loc = max(num_pos * 4.0, 1.0)
    loc_loss = loc_sum / denom_loc if num_pos > 0 else 0.0
    return np.float32(loc_loss), np.float32(conf_loss)


def kernel(loc_pred, conf_pred, targets, alpha, _trace=False):
    B, A, _ = conf_pred.shape
    assert B == 8 and A == 76725
    AP_ = 76800  # pad to 128*600 so every tile covers all 128 partitions
    nc = _get_nc(AP_, 600, 75)

    conf16 = np.zeros((B, AP_, C), dtype=BF16NP)               # [B, AP_, 81]
    conf16[:, :A] = np.asarray(conf_pred, dtype=BF16NP)
    tgt = np.asarray(targets, dtype=np.float32)
    lab_i = np.full((B, AP_), -1, dtype=np.int32)
    lab_i[:, :A] = tgt[:, :, 4].astype(np.int32)               # [B, AP_]
    labq = lab_i // 9
    labr = lab_i - 9 * labq
    xsel = np.take_along_axis(
        conf16, np.maximum(lab_i, 0)[:, :, None], axis=2
    )[:, :, 0]                                                 # [B, AP_] bf16
    cnt = np.bincount(lab_i[lab_i >= 0], minlength=C)[:C]

    aux = np.zeros((B, AP_, 10), dtype=BF16NP)
    aux[:, :A, 0:4] = loc_pred
    aux[:, :A, 4:8] = tgt[:, :, 0:4]
    aux[:, :, 8] = (lab_i > 0)
    aux[:, :, 9] = xsel

    # one-hot planes: qr[.., 0:9] = 1[labq=q], qr[.., 9:18] = 1[labr=r]
    qr = np.zeros((B, AP_, 2 * Q), dtype=BF16NP)
    iq = np.arange(Q)
    qr[:, :, 0:Q] = (labq[:, :, None] == iq)
    qr[:, :, Q : 2 * Q] = (labr[:, :, None] == iq) * (labq[:, :, None] >= 0)

    in_maps = [
        {
            "conf": np.ascontiguousarray(conf16[b]),
            "aux": np.ascontiguousarray(aux[b]),
            "qr": np.ascontiguousarray(qr[b]),
        }
        for b in range(B)
    ]
    res = run_bass_kernel_spmd(nc, in_maps, core_ids=list(range(B)), trace=_trace)
    hists = np.stack([r["outt"][0:Q, 0:Q] for r in res.results])
    locsums = np.stack([r["outt"][0, Q] for r in res.results])
    out = combine_host(hists, locsums, cnt, np.asarray(alpha, dtype=np.float32))
    if _trace:
        return out, res
    return out
